# revision 32
# baseline (speedup 1.0000x reference)
"""Causal self-attention (B=2, T=2048, C=1024, H=16) on 8 TRN2 NeuronCores.

Megatron-style tensor parallelism over heads: each core computes 2 of the 16
heads (Wq/Wk/Wv column-sharded, Wo row-sharded) and produces a partial output
projection; the partials are summed on the host (the all-reduce).

Per-core device dataflow (everything kept transposed so the PE contraction dim
is always the partition dim, no on-device transposes of x needed):
  xP  [128, NTB*NKT*TB] host-packed so each t-block load is one big DMA
  QT/KT/VT = W_locT.T @ xP tiles  (bf16 matmuls, K-tiled over C)
  V tiles   = PE-transpose of VT; head0 stationary [V0|1] (65 cols),
              head1 stationary [0|1|0..|V1] (128 cols) so head1's attention
              output lands on partitions 64-127 directly
  S^T       = K_loc @ Q_loc^T per (batch, head, 128-j-strip, 512-i-block);
              the two heads are packed into PE row groups (contraction D=64,
              tile_position rows 0-63 / 64-127 -> they run concurrently).
              Diagonal strips are ragged: only the unmasked i-suffix is
              computed (moving width w = 512-128q).
  P^T       = exp(S^T / 8) on ACT (no max-subtraction needed: |S|<~3),
              triangular [128,128] mask multiply on the diagonal sub-block
  O^T|s     = [V|1].T @ P^T accumulated over strips (ones col gives sums)
  ylocT     = O^T * (1/s)  (reciprocal + GpSimd partition broadcast)
  yT_part   = Wo_locT.T @ ylocT -> packed DRAM
Host: y = (sum_cores yT_part) + bo, reshape to [B, T, C].

Schedule: PE warmup burst, QKV(0) prologue, then 8 attention phases
(b, ib); QKV(tb+1) and the previous block's output projection are emitted
as filler units interleaved between attention strips so the PE never
idles long enough for the HAM clock gate to re-throttle.
"""

import sys

if "/opt/trn_rl_repo" not in sys.path:
    sys.path.insert(0, "/opt/trn_rl_repo")

import numpy as np

import concourse.bass as bass
import concourse.tile as tile
from concourse import bacc
from concourse import mybir
from concourse.bass_utils import run_bass_kernel_spmd

F32 = mybir.dt.float32
BF16 = mybir.dt.bfloat16
AF = mybir.ActivationFunctionType
ALU = mybir.AluOpType

B, T, C, H = 2, 2048, 1024, 16
D = C // H          # 64
NCORES = 8
HL = H // NCORES    # 2 local heads
CL = C // NCORES    # 128 local channels
BT = B * T          # 4096
TB = 512            # t-block / i-block width
NTB = BT // TB      # 8
NKT = C // 128      # 8 contraction tiles for projections
IB = T // TB        # 4 i-blocks per batch
NJT = T // 128      # 16 j-tiles per batch
VW = 130            # V group width: [V0|1]=65 + [V1|1]=65
NJG = BT // 128     # 32 j-groups total


def build_nc() -> bass.Bass:
    nc = bacc.Bacc()

    x_d = nc.declare_dram_parameter("xP", [128, NTB * NKT * TB], BF16,
                                    isOutput=False)
    wqT_d = nc.declare_dram_parameter("wqT", [128, C], BF16, isOutput=False)
    wkT_d = nc.declare_dram_parameter("wkT", [128, C], BF16, isOutput=False)
    wvT_d = nc.declare_dram_parameter("wvT", [128, C], BF16, isOutput=False)
    woT_d = nc.declare_dram_parameter("woT", [CL, C], BF16, isOutput=False)
    bq_d = nc.declare_dram_parameter("bq", [CL, 1], F32, isOutput=False)
    bk_d = nc.declare_dram_parameter("bk", [CL, 1], F32, isOutput=False)
    bv_d = nc.declare_dram_parameter("bv", [CL, 1], F32, isOutput=False)
    tri_d = nc.declare_dram_parameter("tri", [128, 128], BF16, isOutput=False)
    id_d = nc.declare_dram_parameter("ident", [128, 128], BF16, isOutput=False)
    # output packed: [8 iblocks, 4 co-pairs, 128 p, 1024]
    y_d = nc.declare_dram_parameter("yP", [NTB * 4, 128, 1024], BF16,
                                    isOutput=True)

    with tile.TileContext(nc) as tc:
        with (
            tc.tile_pool(name="const", bufs=1) as const,
            tc.tile_pool(name="work", bufs=2) as work,
            tc.tile_pool(name="psum", bufs=2, space="PSUM") as psum,
        ):
            # ---------------- PE warmup (keep HAM busy during DMA head) -----
            warm_sb = const.tile([128, 512], BF16)
            nc.gpsimd.memset(warm_sb[:, :], 0.0)
            ones_sb = const.tile([1, 64], BF16)
            nc.gpsimd.memset(ones_sb[:, :], 1.0)
            wp = psum.tile([128, 256], F32, tag="acc", bufs=4, name="warm")
            for wi in range(28):
                nc.tensor.matmul(wp[:, :], warm_sb[:, 0:128],
                                 warm_sb[:, 128:384],
                                 start=(wi == 0), stop=(wi == 27))

            # ---------------- constants / persistent state ----------------
            # first x t-block load goes first so the prologue QKV can start
            wq_sb = const.tile([128, C], BF16)
            nc.sync.dma_start(wq_sb[:, :], wqT_d[:, :])
            xt0 = work.tile([128, NKT * TB], BF16, tag="xt", bufs=4,
                            name="xt_0")
            nc.sync.dma_start(xt0[:, 0:NKT * TB // 2], x_d[:, 0:NKT * TB // 2])
            nc.sync.dma_start(xt0[:, NKT * TB // 2:NKT * TB],
                              x_d[:, NKT * TB // 2:NKT * TB])
            wk_sb = const.tile([128, C], BF16)
            wv_sb = const.tile([128, C], BF16)
            nc.sync.dma_start(wk_sb[:, :], wkT_d[:, :])
            nc.sync.dma_start(wv_sb[:, :], wvT_d[:, :])
            bq_sb = const.tile([128, 1], F32)
            nc.sync.dma_start(bq_sb[:, :], bq_d[:, :])
            bk_sb = const.tile([128, 1], F32)
            nc.sync.dma_start(bk_sb[:, :], bk_d[:, :])
            bv_sb = const.tile([128, 1], F32)
            nc.sync.dma_start(bv_sb[:, :], bv_d[:, :])
            id_sb = const.tile([128, 128], BF16)
            nc.sync.dma_start(id_sb[:, :], id_d[:, :])
            tri_sb = const.tile([128, 128], BF16)
            nc.sync.dma_start(tri_sb[:, :], tri_d[:, :])
            wo_sb = const.tile([128, C], BF16)
            nc.sync.dma_start(wo_sb[:, :], woT_d[:, :])
            # head1 rows of woT re-based to partitions 0-63: lets the last
            # block's out-proj contract per head (no ylocT upper-half DMA)
            wo2_sb = const.tile([64, C], BF16)
            nc.sync.dma_start(wo2_sb[:, :], woT_d[64:128, :])

            QT = const.tile([128, BT], BF16)
            KT = const.tile([128, BT], BF16)
            ylocT = const.tile([128, BT], BF16)
            V = const.tile([128, NJG * VW], BF16)
            # zero head1 stationary cols 0..63, set the two ones columns
            v3 = V[:, :].rearrange("p (g c) -> p g c", c=VW)
            nc.gpsimd.memset(v3[:, :, 64:65], 1.0)
            nc.gpsimd.memset(v3[:, :, 129:130], 1.0)

            # ---------------- QKV projection for one t-block ----------------
            xt_tiles = {0: xt0}

            def load_x(tb):
                """Start the DMA for t-block tb's x slab (prefetched one
                phase ahead of the fillers that consume it)."""
                if tb in xt_tiles or tb >= NTB:
                    return
                xt = work.tile([128, NKT * TB], BF16, tag="xt", bufs=4,
                               name=f"xt_{tb}")
                xt_tiles[tb] = xt
                nc.sync.dma_start(
                    xt[:, :], x_d[:, tb * NKT * TB:(tb + 1) * NKT * TB])

            def qkv_units(tb):
                """Returns a list of closures; each emits a chunk of the
                QKV+V-transpose work for t-block tb."""
                tcols = slice(tb * TB, (tb + 1) * TB)
                xt = xt_tiles[tb]
                state = {}

                def u_load():
                    load_x(tb + 2)

                def mk_chain(which, w_sb, b_sb):
                    def u_mm_a():
                        ps = psum.tile([128, TB], F32, tag="acc", bufs=4,
                                       name=f"ps_{which}_{tb}")
                        state[which] = ps
                        for ct in range(4):
                            nc.tensor.matmul(
                                ps[:, :],
                                w_sb[:, ct * 128:(ct + 1) * 128],
                                xt[:, ct * TB:(ct + 1) * TB],
                                start=(ct == 0), stop=False,
                            )

                    def u_mm_b():
                        ps = state[which]
                        for ct in range(4, NKT):
                            nc.tensor.matmul(
                                ps[:, :],
                                w_sb[:, ct * 128:(ct + 1) * 128],
                                xt[:, ct * TB:(ct + 1) * TB],
                                start=False, stop=(ct == NKT - 1),
                            )

                    def u_evac():
                        ps = state[which]
                        if which == "q":
                            nc.vector.tensor_scalar_add(QT[:, tcols], ps[:, :],
                                                        b_sb[:, :])
                        elif which == "k":
                            nc.vector.tensor_scalar_add(KT[:, tcols], ps[:, :],
                                                        b_sb[:, :])
                        else:
                            vt = work.tile([128, TB], BF16, tag="vt", bufs=2,
                                           name=f"vt_{tb}")
                            state["vt"] = vt
                            nc.vector.tensor_scalar_add(vt[:, :], ps[:, :],
                                                        b_sb[:, :])
                    return [u_mm_a, u_mm_b, u_evac]

                def mk_vtrans(q):
                    def u_vt():
                        vt = state["vt"]
                        jg = tb * 4 + q
                        tp = psum.tile([128, 128], BF16, tag="acc", bufs=4,
                                       name=f"tp_{jg}")
                        nc.tensor.transpose(tp[:, :],
                                            vt[:, q * 128:(q + 1) * 128],
                                            id_sb[:, :])
                        off = jg * VW
                        nc.vector.tensor_copy(V[:, off:off + 64], tp[:, 0:64])
                        nc.vector.tensor_copy(V[:, off + 65:off + 129],
                                              tp[:, 64:128])
                    return u_vt

                units = [u_load]
                units += mk_chain("q", wq_sb, bq_sb)
                units += mk_chain("k", wk_sb, bk_sb)
                units += mk_chain("v", wv_sb, bv_sb)
                units += [mk_vtrans(q) for q in range(4)]
                return units

            # -------- output projection units for one finished i-block ------
            blk_state = {}

            def outproj_units(n):
                b, ib = divmod(n, IB)
                i0 = b * T + ib * TB
                icols = slice(i0, i0 + TB)
                state = {}
                units = []

                def mk_co(co):
                    def u_co():
                        yp = psum.tile([128, TB], F32, tag="acc", bufs=4,
                                       name=f"yp_{n}_{co}")
                        if n == 7:
                            # per-head contraction: head1 half comes straight
                            # from yn1 (no ylocT upper-half assembly DMA)
                            nc.tensor.matmul(
                                yp[:, :],
                                wo_sb[0:64, co * 128:(co + 1) * 128],
                                ylocT[0:64, icols],
                                start=True, stop=False,
                            )
                            nc.tensor.matmul(
                                yp[:, :],
                                wo2_sb[:, co * 128:(co + 1) * 128],
                                blk_state[7][:, :],
                                start=False, stop=True,
                            )
                        else:
                            nc.tensor.matmul(
                                yp[:, :],
                                wo_sb[:, co * 128:(co + 1) * 128],
                                ylocT[:, icols],
                                start=True, stop=True,
                            )
                        cp, half = divmod(co, 2)
                        if half == 0:
                            yo = work.tile([128, 1024], BF16, tag="yo", bufs=3,
                                           name=f"yo_{n}_{cp}")
                            state[cp] = yo
                        else:
                            yo = state[cp]
                        hs = slice(half * TB, (half + 1) * TB)
                        if n == 7 and co >= 5:
                            nc.scalar.copy(yo[:, hs], yp[:, :])
                        else:
                            nc.vector.tensor_copy(yo[:, hs], yp[:, :])
                        if half == 1:
                            nc.sync.dma_start(y_d[n * 4 + cp, :, :], yo[:, :])
                    return u_co

                for co in range(8):
                    units.append(mk_co(co))
                return units

            # ---------------- attention for one i-block ---------------------
            def emit_attention(n, fillers):
                """Emit the attention strips for block n=(b*IB+ib), draining
                filler units between strips, then the normalization tail."""
                b, ib = divmod(n, IB)
                i0 = b * T + ib * TB
                icols = slice(i0, i0 + TB)
                njt = 4 * (ib + 1)
                ot0 = psum.tile([128, TB], F32, tag="acc", bufs=4,
                                name=f"ot0_{n}")
                ot1 = psum.tile([128, TB], F32, tag="acc", bufs=4,
                                name=f"ot1_{n}")
                nf = len(fillers)
                fi = 0
                for jt in range(njt):
                    jg = b * NJT + jt
                    q = jt - (njt - 4)          # >=0 on diagonal strips
                    off = 128 * q if q > 0 else 0
                    w = TB - off
                    st = psum.tile([128, 2 * TB], F32, tag="st",
                                   name=f"st_{n}_{jt}")
                    for h in range(HL):
                        hs = slice(h * D, (h + 1) * D)
                        nc.tensor.matmul(
                            st[:, h * TB:(h + 1) * TB],
                            KT[hs, jg * 128:(jg + 1) * 128],
                            QT[hs, icols],
                            start=True, stop=True,
                        )
                    pt = work.tile([128, 2 * TB], BF16, tag="pt", bufs=4,
                                   name=f"pt_{n}_{jt}")
                    nc.scalar.activation(pt[:, :], st[:, :], AF.Exp,
                                         scale=0.125)
                    if q >= 0:
                        nc.vector.tensor_tensor(
                            pt[:, off:off + 128], pt[:, off:off + 128],
                            tri_sb[:, :], ALU.mult)
                        nc.vector.tensor_tensor(
                            pt[:, TB + off:TB + off + 128],
                            pt[:, TB + off:TB + off + 128],
                            tri_sb[:, :], ALU.mult)
                    g = jg * VW
                    nc.tensor.matmul(
                        ot0[0:65, off:TB],
                        V[:, g:g + 65],
                        pt[:, off:TB],
                        start=(jt == 0), stop=(jt == njt - 1),
                    )
                    nc.tensor.matmul(
                        ot1[0:65, off:TB],
                        V[:, g + 65:g + 130],
                        pt[:, TB + off:2 * TB],
                        start=(jt == 0), stop=(jt == njt - 1),
                    )
                    # drain filler units at 1.5x rate (front-loaded, so the
                    # static engine order puts ready projection matmuls ahead
                    # of stall-prone attention ops)
                    want = min(nf, ((jt + 1) * 3 * nf) // (2 * njt))
                    while fi < want:
                        fillers[fi]()
                        fi += 1
                # ---- normalization tail (head1 first: its chain is longer;
                # sum-copies go to ACT to keep DVE free for the next block's
                # mask multiplies) ----
                s1 = work.tile([1, TB], F32, tag="s1", name=f"s1_{n}")
                nc.scalar.copy(s1[:, :], ot1[64:65, :])
                r1 = work.tile([1, TB], F32, tag="r1", name=f"r1_{n}")
                nc.vector.reciprocal_approx_fast(r1[:, :], s1[:, :])
                bc1 = work.tile([64, TB], F32, tag="bc1", name=f"bc1_{n}")
                nc.gpsimd.partition_broadcast(bc1[:, :], r1[0:1, :])
                yn1 = work.tile([64, TB], BF16, tag="yn1", name=f"yn1_{n}")
                nc.vector.tensor_tensor(yn1[:, :], ot1[0:64, :],
                                        bc1[:, :], ALU.mult)
                if n == 7:
                    blk_state[7] = yn1
                else:
                    nc.sync.dma_start(ylocT[64:128, icols], yn1[:, :])
                s0 = work.tile([1, TB], F32, tag="s0", name=f"s0_{n}")
                nc.scalar.copy(s0[:, :], ot0[64:65, :])
                r0 = work.tile([1, TB], F32, tag="r0", name=f"r0_{n}")
                nc.vector.reciprocal_approx_fast(r0[:, :], s0[:, :])
                bc0 = work.tile([64, TB], F32, tag="bc0", name=f"bc0_{n}")
                nc.gpsimd.partition_broadcast(bc0[:, :], r0[0:1, :])
                nc.vector.tensor_tensor(ylocT[0:64, icols], ot0[0:64, :],
                                        bc0[:, :], ALU.mult)
                if n == 7:
                    # keep the PE clock warm while the chain drains so the
                    # final output projection runs at full rate
                    wp2 = psum.tile([128, 512], F32, tag="st",
                                    name="warm_end")
                    for wi in range(12):
                        nc.tensor.matmul(wp2[:, :], warm_sb[:, 0:128],
                                         warm_sb[:, 0:512],
                                         start=(wi == 0), stop=(wi == 11))

            # ---------------- schedule ----------------
            load_x(1)
            for u in qkv_units(0):
                u()
            # out-proj of block m is interleaved into a LATER block's strip
            # loop; short blocks (njt=4) are skipped so their strips aren't
            # head-blocked by not-yet-ready out-proj matmuls.
            pending_op = []          # blocks whose outproj still needs a home
            for n in range(8):
                fillers = []
                if n + 1 < NTB:
                    fillers += qkv_units(n + 1)
                ib = n % IB
                if 4 * (ib + 1) >= 8:
                    for m in pending_op:
                        fillers += outproj_units(m)
                    pending_op = []
                emit_attention(n, fillers)
                pending_op.append(n)
            for m in pending_op:
                for u in outproj_units(m):
                    u()
    nc.compile()
    return nc


def _host_inputs(x, Wq, bq, Wk, bk, Wv, bv, Wo):
    """Build the 8 per-core input maps (host-side layout prep + sharding)."""
    import ml_dtypes
    bf16 = ml_dtypes.bfloat16
    xT = np.ascontiguousarray(x.reshape(BT, C).T.astype(bf16))  # [C, BT]
    # pack so each t-block load is one contiguous [128, NKT*TB] slab:
    # xP[p, tb*4096 + ct*512 + t] = xT[ct*128 + p, tb*512 + t]
    xP = np.ascontiguousarray(
        xT.reshape(NKT, 128, NTB, TB).transpose(1, 2, 0, 3)
        .reshape(128, NTB * NKT * TB))
    tri = np.triu(np.ones((128, 128), np.float32)).astype(bf16)  # [jl, il>=jl]
    ident = np.eye(128, dtype=bf16)

    def wtile(W, rows):
        # device layout: w_sb[p, k*128 + j] = W[rows][j, k*128 + p]
        wT = W[rows, :].T.astype(bf16)                # [C, CL]
        return np.ascontiguousarray(
            wT.reshape(NKT, 128, CL).transpose(1, 0, 2).reshape(128, NKT * CL))

    in_maps = []
    for core in range(NCORES):
        rows = slice(core * CL, (core + 1) * CL)
        in_maps.append({
            "xP": xP,
            "wqT": wtile(Wq, rows),
            "wkT": wtile(Wk, rows),
            "wvT": wtile(Wv, rows),
            "woT": np.ascontiguousarray(Wo[:, rows].T.astype(bf16)),
            "bq": np.ascontiguousarray(bq[rows].reshape(CL, 1).astype(np.float32)),
            "bk": np.ascontiguousarray(bk[rows].reshape(CL, 1).astype(np.float32)),
            "bv": np.ascontiguousarray(bv[rows].reshape(CL, 1).astype(np.float32)),
            "tri": tri,
            "ident": ident,
        })
    return in_maps


_NC_CACHE = None


def _get_nc():
    global _NC_CACHE
    if _NC_CACHE is None:
        _NC_CACHE = build_nc()
    return _NC_CACHE


def _run(inputs, trace=False):
    x = np.asarray(inputs["x"], np.float32)
    in_maps = _host_inputs(
        x,
        np.asarray(inputs["Wq"], np.float32), np.asarray(inputs["bq"], np.float32),
        np.asarray(inputs["Wk"], np.float32), np.asarray(inputs["bk"], np.float32),
        np.asarray(inputs["Wv"], np.float32), np.asarray(inputs["bv"], np.float32),
        np.asarray(inputs["Wo"], np.float32),
    )
    res = run_bass_kernel_spmd(_get_nc(), in_maps, list(range(NCORES)), trace=trace)
    # yP[n, p, cp*?]: [8 iblocks, 4 co-pairs packed] -> y[BT, C]
    yT = np.zeros((NTB * 4, 128, 1024), np.float64)
    for core in range(NCORES):
        yT += res.results[core]["yP"].astype(np.float64)
    # unpack: y[n*512 + t, co*128 + p] = yP[n*4 + cp, p, half*512 + t],
    # co = cp*2 + half
    y = np.zeros((BT, C), np.float64)
    yv = yT.reshape(NTB, 4, 128, 2, TB)        # [n, cp, p, half, t]
    for cp in range(4):
        for half in range(2):
            co = cp * 2 + half
            # [n, p, t] -> y[n*512+t, co*128+p]
            blk = yv[:, cp, :, half, :]        # [NTB, 128, TB]
            y[:, co * 128:(co + 1) * 128] += blk.transpose(0, 2, 1).reshape(
                BT, 128)
    y = y.astype(np.float32) + np.asarray(inputs["bo"], np.float32)
    return y.reshape(B, T, C), res


def kernel(**inputs) -> np.ndarray:
    out, _ = _run(inputs, trace=False)
    return out


def _install_profile_hook():
    """Register the axon NTFF profile hook (the agent image ships the ctypes
    shim in trn_agent_boot but lacks the antenv.axon_hooks module)."""
    import types

    if "antenv.axon_hooks" in sys.modules:
        return
    sys.path.insert(0, "/root/.axon_site")
    from trn_agent_boot.trn_boot import _ntff_profile_via_ctypes

    mod = types.ModuleType("antenv.axon_hooks")
    hook = _ntff_profile_via_ctypes("/opt/axon/libaxon_pjrt.so")
    mod.get_axon_ntff_profile_hook = lambda: hook
    mod.set_axon_ntff_profile_hook = lambda h: None
    sys.modules["antenv.axon_hooks"] = mod
    import antenv

    antenv.axon_hooks = mod
    from concourse import bass_utils as _bu

    _bu.upload_artifacts = lambda tmpdir: tmpdir  # keep artifacts local


def kernel_profiled(**inputs):
    """Returns (output, exec_time_ns) using the NTFF profile of core 0."""
    _install_profile_hook()
    out, res = _run(inputs, trace=True)
    return out, res.exec_time_ns


# revision 33
# speedup vs baseline: 1.0278x; 1.0278x over previous
"""Causal self-attention (B=2, T=2048, C=1024, H=16) on 8 TRN2 NeuronCores.

Megatron-style tensor parallelism over heads: each core computes 2 of the 16
heads (Wq/Wk/Wv column-sharded, Wo row-sharded) and produces a partial output
projection; the partials are summed on the host (the all-reduce).

Per-core device dataflow (everything kept transposed so the PE contraction dim
is always the partition dim, no on-device transposes of x needed):
  xP  [128, NTB*NKT*TB] host-packed so each t-block load is one big DMA
  QT/KT/VT = W_locT.T @ xP tiles  (bf16 matmuls, K-tiled over C)
  V tiles   = PE-transpose of VT; head0 stationary [V0|1] (65 cols),
              head1 stationary [0|1|0..|V1] (128 cols) so head1's attention
              output lands on partitions 64-127 directly
  S^T       = K_loc @ Q_loc^T per (batch, head, 128-j-strip, 512-i-block);
              the two heads are packed into PE row groups (contraction D=64,
              tile_position rows 0-63 / 64-127 -> they run concurrently).
              Diagonal strips are ragged: only the unmasked i-suffix is
              computed (moving width w = 512-128q).
  P^T       = exp(S^T / 8) on ACT (no max-subtraction needed: |S|<~3),
              triangular [128,128] mask multiply on the diagonal sub-block
  O^T|s     = [V|1].T @ P^T accumulated over strips (ones col gives sums)
  ylocT     = O^T * (1/s)  (reciprocal + GpSimd partition broadcast)
  yT_part   = Wo_locT.T @ ylocT -> packed DRAM
Host: y = (sum_cores yT_part) + bo, reshape to [B, T, C].

Schedule: PE warmup burst, QKV(0) prologue, then 8 attention phases
(b, ib); QKV(tb+1) and the previous block's output projection are emitted
as filler units interleaved between attention strips so the PE never
idles long enough for the HAM clock gate to re-throttle.
"""

import sys

if "/opt/trn_rl_repo" not in sys.path:
    sys.path.insert(0, "/opt/trn_rl_repo")

import numpy as np

import concourse.bass as bass
import concourse.tile as tile
from concourse import bacc
from concourse import mybir
from concourse.bass_utils import run_bass_kernel_spmd

F32 = mybir.dt.float32
BF16 = mybir.dt.bfloat16
AF = mybir.ActivationFunctionType
ALU = mybir.AluOpType

B, T, C, H = 2, 2048, 1024, 16
D = C // H          # 64
NCORES = 8
HL = H // NCORES    # 2 local heads
CL = C // NCORES    # 128 local channels
BT = B * T          # 4096
TB = 512            # t-block / i-block width
NTB = BT // TB      # 8
NKT = C // 128      # 8 contraction tiles for projections
IB = T // TB        # 4 i-blocks per batch
NJT = T // 128      # 16 j-tiles per batch
VW = 130            # V group width: [V0|1]=65 + [V1|1]=65
NJG = BT // 128     # 32 j-groups total


def build_nc() -> bass.Bass:
    nc = bacc.Bacc()

    x_d = nc.declare_dram_parameter("xP", [128, NTB * NKT * TB], BF16,
                                    isOutput=False)
    wqT_d = nc.declare_dram_parameter("wqT", [128, C], BF16, isOutput=False)
    wkT_d = nc.declare_dram_parameter("wkT", [128, C], BF16, isOutput=False)
    wvT_d = nc.declare_dram_parameter("wvT", [128, C], BF16, isOutput=False)
    woT_d = nc.declare_dram_parameter("woT", [CL, C], BF16, isOutput=False)
    bq_d = nc.declare_dram_parameter("bq", [CL, 1], F32, isOutput=False)
    bk_d = nc.declare_dram_parameter("bk", [CL, 1], F32, isOutput=False)
    bv_d = nc.declare_dram_parameter("bv", [CL, 1], F32, isOutput=False)
    tri_d = nc.declare_dram_parameter("tri", [128, 128], BF16, isOutput=False)
    id_d = nc.declare_dram_parameter("ident", [128, 128], BF16, isOutput=False)
    # output packed: [8 iblocks, 4 co-pairs, 128 p, 1024]
    y_d = nc.declare_dram_parameter("yP", [NTB * 4, 128, 1024], BF16,
                                    isOutput=True)

    with tile.TileContext(nc) as tc:
        with (
            tc.tile_pool(name="const", bufs=1) as const,
            tc.tile_pool(name="work", bufs=2) as work,
            tc.tile_pool(name="psum", bufs=2, space="PSUM") as psum,
        ):
            # ---------------- PE warmup (keep HAM busy during DMA head) -----
            warm_sb = const.tile([128, 512], BF16)
            nc.gpsimd.memset(warm_sb[:, :], 0.0)
            ones_sb = const.tile([1, 64], BF16)
            nc.gpsimd.memset(ones_sb[:, :], 1.0)
            wp = psum.tile([128, 256], F32, tag="acc", bufs=4, name="warm")
            for wi in range(28):
                nc.tensor.matmul(wp[:, :], warm_sb[:, 0:128],
                                 warm_sb[:, 128:384],
                                 start=(wi == 0), stop=(wi == 27))

            # ---------------- constants / persistent state ----------------
            # first x t-block load goes first so the prologue QKV can start
            wq_sb = const.tile([128, C], BF16)
            nc.sync.dma_start(wq_sb[:, :], wqT_d[:, :])
            xt0 = work.tile([128, NKT * TB], BF16, tag="xt", bufs=4,
                            name="xt_0")
            nc.sync.dma_start(xt0[:, 0:NKT * TB // 2], x_d[:, 0:NKT * TB // 2])
            nc.sync.dma_start(xt0[:, NKT * TB // 2:NKT * TB],
                              x_d[:, NKT * TB // 2:NKT * TB])
            wk_sb = const.tile([128, C], BF16)
            wv_sb = const.tile([128, C], BF16)
            nc.sync.dma_start(wk_sb[:, :], wkT_d[:, :])
            nc.sync.dma_start(wv_sb[:, :], wvT_d[:, :])
            bq_sb = const.tile([128, 1], F32)
            nc.sync.dma_start(bq_sb[:, :], bq_d[:, :])
            bk_sb = const.tile([128, 1], F32)
            nc.sync.dma_start(bk_sb[:, :], bk_d[:, :])
            bv_sb = const.tile([128, 1], F32)
            nc.sync.dma_start(bv_sb[:, :], bv_d[:, :])
            id_sb = const.tile([128, 128], BF16)
            nc.sync.dma_start(id_sb[:, :], id_d[:, :])
            tri_sb = const.tile([128, 128], BF16)
            nc.sync.dma_start(tri_sb[:, :], tri_d[:, :])
            wo_sb = const.tile([128, C], BF16)
            nc.sync.dma_start(wo_sb[:, :], woT_d[:, :])
            # head1 rows of woT re-based to partitions 0-63: lets the last
            # block's out-proj contract per head (no ylocT upper-half DMA)
            wo2_sb = const.tile([64, C], BF16)
            nc.sync.dma_start(wo2_sb[:, :], woT_d[64:128, :])

            QT = const.tile([128, BT], BF16)
            KT = const.tile([128, BT], BF16)
            ylocT = const.tile([128, BT], BF16)
            V = const.tile([128, NJG * VW], BF16)
            # zero head1 stationary cols 0..63, set the two ones columns
            v3 = V[:, :].rearrange("p (g c) -> p g c", c=VW)
            nc.gpsimd.memset(v3[:, :, 64:65], 1.0)
            nc.gpsimd.memset(v3[:, :, 129:130], 1.0)

            # ---------------- QKV projection for one t-block ----------------
            xt_tiles = {0: xt0}

            def load_x(tb):
                """Start the DMA for t-block tb's x slab (prefetched one
                phase ahead of the fillers that consume it)."""
                if tb in xt_tiles or tb >= NTB:
                    return
                xt = work.tile([128, NKT * TB], BF16, tag="xt", bufs=4,
                               name=f"xt_{tb}")
                xt_tiles[tb] = xt
                nc.sync.dma_start(
                    xt[:, :], x_d[:, tb * NKT * TB:(tb + 1) * NKT * TB])

            def qkv_units(tb):
                """Returns a list of closures; each emits a chunk of the
                QKV+V-transpose work for t-block tb."""
                tcols = slice(tb * TB, (tb + 1) * TB)
                xt = xt_tiles[tb]
                state = {}

                def u_load():
                    load_x(tb + 2)

                def mk_chain(which, w_sb, b_sb):
                    def u_mm_a():
                        ps = psum.tile([128, TB], F32, tag="acc", bufs=4,
                                       name=f"ps_{which}_{tb}")
                        state[which] = ps
                        for ct in range(4):
                            nc.tensor.matmul(
                                ps[:, :],
                                w_sb[:, ct * 128:(ct + 1) * 128],
                                xt[:, ct * TB:(ct + 1) * TB],
                                start=(ct == 0), stop=False,
                            )

                    def u_mm_b():
                        ps = state[which]
                        for ct in range(4, NKT):
                            nc.tensor.matmul(
                                ps[:, :],
                                w_sb[:, ct * 128:(ct + 1) * 128],
                                xt[:, ct * TB:(ct + 1) * TB],
                                start=False, stop=(ct == NKT - 1),
                            )

                    def u_evac():
                        ps = state[which]
                        if which == "q":
                            nc.vector.tensor_scalar_add(QT[:, tcols], ps[:, :],
                                                        b_sb[:, :])
                        elif which == "k":
                            nc.vector.tensor_scalar_add(KT[:, tcols], ps[:, :],
                                                        b_sb[:, :])
                        else:
                            vt = work.tile([128, TB], BF16, tag="vt", bufs=2,
                                           name=f"vt_{tb}")
                            state["vt"] = vt
                            nc.vector.tensor_scalar_add(vt[:, :], ps[:, :],
                                                        b_sb[:, :])
                    return [u_mm_a, u_mm_b, u_evac]

                def mk_vtrans(q):
                    def u_vt():
                        vt = state["vt"]
                        jg = tb * 4 + q
                        tp = psum.tile([128, 128], BF16, tag="acc", bufs=4,
                                       name=f"tp_{jg}")
                        nc.tensor.transpose(tp[:, :],
                                            vt[:, q * 128:(q + 1) * 128],
                                            id_sb[:, :])
                        off = jg * VW
                        nc.vector.tensor_copy(V[:, off:off + 64], tp[:, 0:64])
                        nc.vector.tensor_copy(V[:, off + 65:off + 129],
                                              tp[:, 64:128])
                    return u_vt

                units = [u_load]
                units += mk_chain("q", wq_sb, bq_sb)
                units += mk_chain("k", wk_sb, bk_sb)
                units += mk_chain("v", wv_sb, bv_sb)
                units += [mk_vtrans(q) for q in range(4)]
                return units

            # -------- output projection units for one finished i-block ------
            blk_state = {}

            def outproj_units(n):
                b, ib = divmod(n, IB)
                i0 = b * T + ib * TB
                icols = slice(i0, i0 + TB)
                state = {}
                units = []

                def mk_co(co):
                    def u_co():
                        yp = psum.tile([128, TB], F32, tag="acc", bufs=4,
                                       name=f"yp_{n}_{co}")
                        if n == 7:
                            # per-head contraction: head1 half comes straight
                            # from yn1 (no ylocT upper-half assembly DMA)
                            nc.tensor.matmul(
                                yp[:, :],
                                wo_sb[0:64, co * 128:(co + 1) * 128],
                                ylocT[0:64, icols],
                                start=True, stop=False,
                            )
                            nc.tensor.matmul(
                                yp[:, :],
                                wo2_sb[:, co * 128:(co + 1) * 128],
                                blk_state[7][:, :],
                                start=False, stop=True,
                            )
                        else:
                            nc.tensor.matmul(
                                yp[:, :],
                                wo_sb[:, co * 128:(co + 1) * 128],
                                ylocT[:, icols],
                                start=True, stop=True,
                            )
                        cp, half = divmod(co, 2)
                        if half == 0:
                            yo = work.tile([128, 1024], BF16, tag="yo", bufs=3,
                                           name=f"yo_{n}_{cp}")
                            state[cp] = yo
                        else:
                            yo = state[cp]
                        hs = slice(half * TB, (half + 1) * TB)
                        if n == 7 and co >= 5:
                            nc.scalar.copy(yo[:, hs], yp[:, :])
                        else:
                            nc.vector.tensor_copy(yo[:, hs], yp[:, :])
                        if half == 1:
                            nc.sync.dma_start(y_d[n * 4 + cp, :, :], yo[:, :])
                    return u_co

                for co in range(8):
                    units.append(mk_co(co))
                return units

            # ---------------- attention for one i-block ---------------------
            def emit_attention(n, fillers):
                """Emit the attention strips for block n=(b*IB+ib), draining
                filler units between strips, then the normalization tail."""
                b, ib = divmod(n, IB)
                i0 = b * T + ib * TB
                icols = slice(i0, i0 + TB)
                njt = 4 * (ib + 1)
                ot0 = psum.tile([128, TB], F32, tag="acc", bufs=4,
                                name=f"ot0_{n}")
                ot1 = psum.tile([128, TB], F32, tag="acc", bufs=4,
                                name=f"ot1_{n}")
                nf = len(fillers)
                fi = 0
                for jt in range(njt):
                    jg = b * NJT + jt
                    q = jt - (njt - 4)          # >=0 on diagonal strips
                    off = 128 * q if q > 0 else 0
                    w = TB - off
                    st = psum.tile([128, 2 * TB], F32, tag="st",
                                   name=f"st_{n}_{jt}")
                    for h in range(HL):
                        hs = slice(h * D, (h + 1) * D)
                        nc.tensor.matmul(
                            st[:, h * TB:(h + 1) * TB],
                            KT[hs, jg * 128:(jg + 1) * 128],
                            QT[hs, icols],
                            start=True, stop=True,
                        )
                    pt = work.tile([128, 2 * TB], BF16, tag="pt", bufs=4,
                                   name=f"pt_{n}_{jt}")
                    nc.scalar.activation(pt[:, :], st[:, :], AF.Exp,
                                         scale=0.125)
                    if q >= 0:
                        nc.vector.tensor_tensor(
                            pt[:, off:off + 128], pt[:, off:off + 128],
                            tri_sb[:, :], ALU.mult)
                        nc.vector.tensor_tensor(
                            pt[:, TB + off:TB + off + 128],
                            pt[:, TB + off:TB + off + 128],
                            tri_sb[:, :], ALU.mult)
                    g = jg * VW
                    nc.tensor.matmul(
                        ot0[0:65, off:TB],
                        V[:, g:g + 65],
                        pt[:, off:TB],
                        start=(jt == 0), stop=(jt == njt - 1),
                    )
                    nc.tensor.matmul(
                        ot1[0:65, off:TB],
                        V[:, g + 65:g + 130],
                        pt[:, TB + off:2 * TB],
                        start=(jt == 0), stop=(jt == njt - 1),
                    )
                    # drain filler units at 1.5x rate (front-loaded, so the
                    # static engine order puts ready projection matmuls ahead
                    # of stall-prone attention ops)
                    want = min(nf, ((jt + 1) * 3 * nf) // (2 * njt))
                    while fi < want:
                        fillers[fi]()
                        fi += 1
                # ---- normalization tail (head1 first: its chain is longer;
                # sum-copies go to ACT to keep DVE free for the next block's
                # mask multiplies) ----
                s1 = work.tile([1, TB], F32, tag="s1", name=f"s1_{n}")
                nc.vector.tensor_copy(s1[:, :], ot1[64:65, :])
                r1 = work.tile([1, TB], F32, tag="r1", name=f"r1_{n}")
                nc.vector.reciprocal_approx_fast(r1[:, :], s1[:, :])
                bc1 = work.tile([64, TB], F32, tag="bc1", name=f"bc1_{n}")
                nc.gpsimd.partition_broadcast(bc1[:, :], r1[0:1, :])
                yn1 = work.tile([64, TB], BF16, tag="yn1", name=f"yn1_{n}")
                nc.vector.tensor_tensor(yn1[:, :], ot1[0:64, :],
                                        bc1[:, :], ALU.mult)
                if n == 7:
                    blk_state[7] = yn1
                else:
                    nc.sync.dma_start(ylocT[64:128, icols], yn1[:, :])
                s0 = work.tile([1, TB], F32, tag="s0", name=f"s0_{n}")
                nc.vector.tensor_copy(s0[:, :], ot0[64:65, :])
                r0 = work.tile([1, TB], F32, tag="r0", name=f"r0_{n}")
                nc.vector.reciprocal_approx_fast(r0[:, :], s0[:, :])
                bc0 = work.tile([64, TB], F32, tag="bc0", name=f"bc0_{n}")
                nc.gpsimd.partition_broadcast(bc0[:, :], r0[0:1, :])
                nc.vector.tensor_tensor(ylocT[0:64, icols], ot0[0:64, :],
                                        bc0[:, :], ALU.mult)
                if n == 7:
                    # keep the PE clock warm while the chain drains so the
                    # final output projection runs at full rate
                    wp2 = psum.tile([128, 512], F32, tag="st",
                                    name="warm_end")
                    for wi in range(12):
                        nc.tensor.matmul(wp2[:, :], warm_sb[:, 0:128],
                                         warm_sb[:, 0:512],
                                         start=(wi == 0), stop=(wi == 11))

            # ---------------- schedule ----------------
            load_x(1)
            for u in qkv_units(0):
                u()
            # out-proj of block m is interleaved into a LATER block's strip
            # loop; short blocks (njt=4) are skipped so their strips aren't
            # head-blocked by not-yet-ready out-proj matmuls.
            pending_op = []          # blocks whose outproj still needs a home
            for n in range(8):
                fillers = []
                if n + 1 < NTB:
                    fillers += qkv_units(n + 1)
                ib = n % IB
                if 4 * (ib + 1) >= 8:
                    for m in pending_op:
                        fillers += outproj_units(m)
                    pending_op = []
                emit_attention(n, fillers)
                pending_op.append(n)
            for m in pending_op:
                for u in outproj_units(m):
                    u()
    nc.compile()
    return nc


def _host_inputs(x, Wq, bq, Wk, bk, Wv, bv, Wo):
    """Build the 8 per-core input maps (host-side layout prep + sharding)."""
    import ml_dtypes
    bf16 = ml_dtypes.bfloat16
    xT = np.ascontiguousarray(x.reshape(BT, C).T.astype(bf16))  # [C, BT]
    # pack so each t-block load is one contiguous [128, NKT*TB] slab:
    # xP[p, tb*4096 + ct*512 + t] = xT[ct*128 + p, tb*512 + t]
    xP = np.ascontiguousarray(
        xT.reshape(NKT, 128, NTB, TB).transpose(1, 2, 0, 3)
        .reshape(128, NTB * NKT * TB))
    tri = np.triu(np.ones((128, 128), np.float32)).astype(bf16)  # [jl, il>=jl]
    ident = np.eye(128, dtype=bf16)

    def wtile(W, rows):
        # device layout: w_sb[p, k*128 + j] = W[rows][j, k*128 + p]
        wT = W[rows, :].T.astype(bf16)                # [C, CL]
        return np.ascontiguousarray(
            wT.reshape(NKT, 128, CL).transpose(1, 0, 2).reshape(128, NKT * CL))

    in_maps = []
    for core in range(NCORES):
        rows = slice(core * CL, (core + 1) * CL)
        in_maps.append({
            "xP": xP,
            "wqT": wtile(Wq, rows),
            "wkT": wtile(Wk, rows),
            "wvT": wtile(Wv, rows),
            "woT": np.ascontiguousarray(Wo[:, rows].T.astype(bf16)),
            "bq": np.ascontiguousarray(bq[rows].reshape(CL, 1).astype(np.float32)),
            "bk": np.ascontiguousarray(bk[rows].reshape(CL, 1).astype(np.float32)),
            "bv": np.ascontiguousarray(bv[rows].reshape(CL, 1).astype(np.float32)),
            "tri": tri,
            "ident": ident,
        })
    return in_maps


_NC_CACHE = None


def _get_nc():
    global _NC_CACHE
    if _NC_CACHE is None:
        _NC_CACHE = build_nc()
    return _NC_CACHE


def _run(inputs, trace=False):
    x = np.asarray(inputs["x"], np.float32)
    in_maps = _host_inputs(
        x,
        np.asarray(inputs["Wq"], np.float32), np.asarray(inputs["bq"], np.float32),
        np.asarray(inputs["Wk"], np.float32), np.asarray(inputs["bk"], np.float32),
        np.asarray(inputs["Wv"], np.float32), np.asarray(inputs["bv"], np.float32),
        np.asarray(inputs["Wo"], np.float32),
    )
    res = run_bass_kernel_spmd(_get_nc(), in_maps, list(range(NCORES)), trace=trace)
    # yP[n, p, cp*?]: [8 iblocks, 4 co-pairs packed] -> y[BT, C]
    yT = np.zeros((NTB * 4, 128, 1024), np.float64)
    for core in range(NCORES):
        yT += res.results[core]["yP"].astype(np.float64)
    # unpack: y[n*512 + t, co*128 + p] = yP[n*4 + cp, p, half*512 + t],
    # co = cp*2 + half
    y = np.zeros((BT, C), np.float64)
    yv = yT.reshape(NTB, 4, 128, 2, TB)        # [n, cp, p, half, t]
    for cp in range(4):
        for half in range(2):
            co = cp * 2 + half
            # [n, p, t] -> y[n*512+t, co*128+p]
            blk = yv[:, cp, :, half, :]        # [NTB, 128, TB]
            y[:, co * 128:(co + 1) * 128] += blk.transpose(0, 2, 1).reshape(
                BT, 128)
    y = y.astype(np.float32) + np.asarray(inputs["bo"], np.float32)
    return y.reshape(B, T, C), res


def kernel(**inputs) -> np.ndarray:
    out, _ = _run(inputs, trace=False)
    return out


def _install_profile_hook():
    """Register the axon NTFF profile hook (the agent image ships the ctypes
    shim in trn_agent_boot but lacks the antenv.axon_hooks module)."""
    import types

    if "antenv.axon_hooks" in sys.modules:
        return
    sys.path.insert(0, "/root/.axon_site")
    from trn_agent_boot.trn_boot import _ntff_profile_via_ctypes

    mod = types.ModuleType("antenv.axon_hooks")
    hook = _ntff_profile_via_ctypes("/opt/axon/libaxon_pjrt.so")
    mod.get_axon_ntff_profile_hook = lambda: hook
    mod.set_axon_ntff_profile_hook = lambda h: None
    sys.modules["antenv.axon_hooks"] = mod
    import antenv

    antenv.axon_hooks = mod
    from concourse import bass_utils as _bu

    _bu.upload_artifacts = lambda tmpdir: tmpdir  # keep artifacts local


def kernel_profiled(**inputs):
    """Returns (output, exec_time_ns) using the NTFF profile of core 0."""
    _install_profile_hook()
    out, res = _run(inputs, trace=True)
    return out, res.exec_time_ns


# revision 34
# speedup vs baseline: 1.0443x; 1.0161x over previous
"""Causal self-attention (B=2, T=2048, C=1024, H=16) on 8 TRN2 NeuronCores.

Megatron-style tensor parallelism over heads: each core computes 2 of the 16
heads (Wq/Wk/Wv column-sharded, Wo row-sharded) and produces a partial output
projection; the partials are summed on the host (the all-reduce).

Per-core device dataflow (everything kept transposed so the PE contraction dim
is always the partition dim, no on-device transposes of x needed):
  xP  [128, NTB*NKT*TB] host-packed so each t-block load is one big DMA
  QT/KT/VT = W_locT.T @ xP tiles  (bf16 matmuls, K-tiled over C)
  V tiles   = PE-transpose of VT; head0 stationary [V0|1] (65 cols),
              head1 stationary [0|1|0..|V1] (128 cols) so head1's attention
              output lands on partitions 64-127 directly
  S^T       = K_loc @ Q_loc^T per (batch, head, 128-j-strip, 512-i-block);
              the two heads are packed into PE row groups (contraction D=64,
              tile_position rows 0-63 / 64-127 -> they run concurrently).
              Diagonal strips are ragged: only the unmasked i-suffix is
              computed (moving width w = 512-128q).
  P^T       = exp(S^T / 8) on ACT (no max-subtraction needed: |S|<~3),
              triangular [128,128] mask multiply on the diagonal sub-block
  O^T|s     = [V|1].T @ P^T accumulated over strips (ones col gives sums)
  ylocT     = O^T * (1/s)  (reciprocal + GpSimd partition broadcast)
  yT_part   = Wo_locT.T @ ylocT -> packed DRAM
Host: y = (sum_cores yT_part) + bo, reshape to [B, T, C].

Schedule: PE warmup burst, QKV(0) prologue, then 8 attention phases
(b, ib); QKV(tb+1) and the previous block's output projection are emitted
as filler units interleaved between attention strips so the PE never
idles long enough for the HAM clock gate to re-throttle.
"""

import sys

if "/opt/trn_rl_repo" not in sys.path:
    sys.path.insert(0, "/opt/trn_rl_repo")

import numpy as np

import concourse.bass as bass
import concourse.tile as tile
from concourse import bacc
from concourse import mybir
from concourse.bass_utils import run_bass_kernel_spmd

F32 = mybir.dt.float32
BF16 = mybir.dt.bfloat16
AF = mybir.ActivationFunctionType
ALU = mybir.AluOpType

B, T, C, H = 2, 2048, 1024, 16
D = C // H          # 64
NCORES = 8
HL = H // NCORES    # 2 local heads
CL = C // NCORES    # 128 local channels
BT = B * T          # 4096
TB = 512            # t-block / i-block width
NTB = BT // TB      # 8
NKT = C // 128      # 8 contraction tiles for projections
IB = T // TB        # 4 i-blocks per batch
NJT = T // 128      # 16 j-tiles per batch
VW = 130            # V group width: [V0|1]=65 + [V1|1]=65
NJG = BT // 128     # 32 j-groups total


def build_nc() -> bass.Bass:
    nc = bacc.Bacc()

    x_d = nc.declare_dram_parameter("xP", [128, NTB * NKT * TB], BF16,
                                    isOutput=False)
    wqT_d = nc.declare_dram_parameter("wqT", [128, C], BF16, isOutput=False)
    wkT_d = nc.declare_dram_parameter("wkT", [128, C], BF16, isOutput=False)
    wvT_d = nc.declare_dram_parameter("wvT", [128, C], BF16, isOutput=False)
    woT_d = nc.declare_dram_parameter("woT", [CL, C], BF16, isOutput=False)
    bq_d = nc.declare_dram_parameter("bq", [CL, 1], F32, isOutput=False)
    bk_d = nc.declare_dram_parameter("bk", [CL, 1], F32, isOutput=False)
    bv_d = nc.declare_dram_parameter("bv", [CL, 1], F32, isOutput=False)
    tri_d = nc.declare_dram_parameter("tri", [128, 128], BF16, isOutput=False)
    id_d = nc.declare_dram_parameter("ident", [128, 128], BF16, isOutput=False)
    # output packed: [8 iblocks, 4 co-pairs, 128 p, 1024]
    y_d = nc.declare_dram_parameter("yP", [NTB * 4, 128, 1024], BF16,
                                    isOutput=True)

    with tile.TileContext(nc) as tc:
        with (
            tc.tile_pool(name="const", bufs=1) as const,
            tc.tile_pool(name="work", bufs=2) as work,
            tc.tile_pool(name="psum", bufs=2, space="PSUM") as psum,
        ):
            # ---------------- PE warmup (keep HAM busy during DMA head) -----
            warm_sb = const.tile([128, 512], BF16)
            nc.gpsimd.memset(warm_sb[:, :], 0.0)
            ones_sb = const.tile([1, 64], BF16)
            nc.gpsimd.memset(ones_sb[:, :], 1.0)
            wp = psum.tile([128, 256], F32, tag="acc", bufs=4, name="warm")
            for wi in range(28):
                nc.tensor.matmul(wp[:, :], warm_sb[:, 0:128],
                                 warm_sb[:, 128:384],
                                 start=(wi == 0), stop=(wi == 27))

            # ---------------- constants / persistent state ----------------
            # first x t-block load goes first so the prologue QKV can start
            wq_sb = const.tile([128, C], BF16)
            nc.sync.dma_start(wq_sb[:, :], wqT_d[:, :])
            xt0 = work.tile([128, NKT * TB], BF16, tag="xt", bufs=4,
                            name="xt_0")
            nc.sync.dma_start(xt0[:, 0:NKT * TB // 2], x_d[:, 0:NKT * TB // 2])
            nc.sync.dma_start(xt0[:, NKT * TB // 2:NKT * TB],
                              x_d[:, NKT * TB // 2:NKT * TB])
            wk_sb = const.tile([128, C], BF16)
            wv_sb = const.tile([128, C], BF16)
            nc.sync.dma_start(wk_sb[:, :], wkT_d[:, :])
            nc.sync.dma_start(wv_sb[:, :], wvT_d[:, :])
            bq_sb = const.tile([128, 1], F32)
            nc.sync.dma_start(bq_sb[:, :], bq_d[:, :])
            bk_sb = const.tile([128, 1], F32)
            nc.sync.dma_start(bk_sb[:, :], bk_d[:, :])
            bv_sb = const.tile([128, 1], F32)
            nc.sync.dma_start(bv_sb[:, :], bv_d[:, :])
            id_sb = const.tile([128, 128], BF16)
            nc.sync.dma_start(id_sb[:, :], id_d[:, :])
            tri_sb = const.tile([128, 128], BF16)
            nc.sync.dma_start(tri_sb[:, :], tri_d[:, :])
            wo_sb = const.tile([128, C], BF16)
            nc.sync.dma_start(wo_sb[:, :], woT_d[:, :])
            # head1 rows of woT re-based to partitions 0-63: lets the last
            # block's out-proj contract per head (no ylocT upper-half DMA)
            wo2_sb = const.tile([64, C], BF16)
            nc.sync.dma_start(wo2_sb[:, :], woT_d[64:128, :])

            QT = const.tile([128, BT], BF16)
            KT = const.tile([128, BT], BF16)
            ylocT = const.tile([128, BT], BF16)
            V = const.tile([128, NJG * VW], BF16)
            # zero head1 stationary cols 0..63, set the two ones columns
            v3 = V[:, :].rearrange("p (g c) -> p g c", c=VW)
            nc.gpsimd.memset(v3[:, :, 64:65], 1.0)
            nc.gpsimd.memset(v3[:, :, 129:130], 1.0)

            # ---------------- QKV projection for one t-block ----------------
            xt_tiles = {0: xt0}

            def load_x(tb):
                """Start the DMA for t-block tb's x slab (prefetched one
                phase ahead of the fillers that consume it)."""
                if tb in xt_tiles or tb >= NTB:
                    return
                xt = work.tile([128, NKT * TB], BF16, tag="xt", bufs=4,
                               name=f"xt_{tb}")
                xt_tiles[tb] = xt
                nc.sync.dma_start(
                    xt[:, :], x_d[:, tb * NKT * TB:(tb + 1) * NKT * TB])

            def qkv_units(tb):
                """Returns a list of closures; each emits a chunk of the
                QKV+V-transpose work for t-block tb."""
                tcols = slice(tb * TB, (tb + 1) * TB)
                xt = xt_tiles[tb]
                state = {}

                def u_load():
                    load_x(tb + 2)

                def mk_chain(which, w_sb, b_sb):
                    def u_mm_a():
                        ps = psum.tile([128, TB], F32, tag="acc", bufs=4,
                                       name=f"ps_{which}_{tb}")
                        state[which] = ps
                        for ct in range(4):
                            nc.tensor.matmul(
                                ps[:, :],
                                w_sb[:, ct * 128:(ct + 1) * 128],
                                xt[:, ct * TB:(ct + 1) * TB],
                                start=(ct == 0), stop=False,
                            )

                    def u_mm_b():
                        ps = state[which]
                        for ct in range(4, NKT):
                            nc.tensor.matmul(
                                ps[:, :],
                                w_sb[:, ct * 128:(ct + 1) * 128],
                                xt[:, ct * TB:(ct + 1) * TB],
                                start=False, stop=(ct == NKT - 1),
                            )

                    def u_evac():
                        ps = state[which]
                        if which == "q":
                            nc.vector.tensor_scalar_add(QT[:, tcols], ps[:, :],
                                                        b_sb[:, :])
                        elif which == "k":
                            nc.vector.tensor_scalar_add(KT[:, tcols], ps[:, :],
                                                        b_sb[:, :])
                        else:
                            vt = work.tile([128, TB], BF16, tag="vt", bufs=2,
                                           name=f"vt_{tb}")
                            state["vt"] = vt
                            nc.vector.tensor_scalar_add(vt[:, :], ps[:, :],
                                                        b_sb[:, :])
                    return [u_mm_a, u_mm_b, u_evac]

                def mk_vtrans(q):
                    def u_vt():
                        vt = state["vt"]
                        jg = tb * 4 + q
                        tp = psum.tile([128, 128], BF16, tag="acc", bufs=4,
                                       name=f"tp_{jg}")
                        nc.tensor.transpose(tp[:, :],
                                            vt[:, q * 128:(q + 1) * 128],
                                            id_sb[:, :])
                        off = jg * VW
                        nc.vector.tensor_copy(V[:, off:off + 64], tp[:, 0:64])
                        nc.vector.tensor_copy(V[:, off + 65:off + 129],
                                              tp[:, 64:128])
                    return u_vt

                units = [u_load]
                units += mk_chain("q", wq_sb, bq_sb)
                units += mk_chain("k", wk_sb, bk_sb)
                units += mk_chain("v", wv_sb, bv_sb)
                units += [mk_vtrans(q) for q in range(4)]
                return units

            # -------- output projection units for one finished i-block ------
            blk_state = {}

            def outproj_units(n):
                b, ib = divmod(n, IB)
                i0 = b * T + ib * TB
                icols = slice(i0, i0 + TB)
                state = {}
                units = []

                def mk_co(co):
                    def u_co():
                        yp = psum.tile([128, TB], F32, tag="acc", bufs=4,
                                       name=f"yp_{n}_{co}")
                        if n == 7:
                            # per-head contraction: head1 half comes straight
                            # from yn1 (no ylocT upper-half assembly DMA)
                            nc.tensor.matmul(
                                yp[:, :],
                                wo_sb[0:64, co * 128:(co + 1) * 128],
                                ylocT[0:64, icols],
                                start=True, stop=False,
                            )
                            nc.tensor.matmul(
                                yp[:, :],
                                wo2_sb[:, co * 128:(co + 1) * 128],
                                blk_state[7][:, :],
                                start=False, stop=True,
                            )
                        else:
                            nc.tensor.matmul(
                                yp[:, :],
                                wo_sb[:, co * 128:(co + 1) * 128],
                                ylocT[:, icols],
                                start=True, stop=True,
                            )
                        cp, half = divmod(co, 2)
                        if half == 0:
                            yo = work.tile([128, 1024], BF16, tag="yo", bufs=3,
                                           name=f"yo_{n}_{cp}")
                            state[cp] = yo
                        else:
                            yo = state[cp]
                        hs = slice(half * TB, (half + 1) * TB)
                        if n == 7 and co >= 5:
                            nc.scalar.copy(yo[:, hs], yp[:, :])
                        else:
                            nc.vector.tensor_copy(yo[:, hs], yp[:, :])
                        if half == 1:
                            nc.sync.dma_start(y_d[n * 4 + cp, :, :], yo[:, :])
                    return u_co

                for co in range(8):
                    units.append(mk_co(co))
                return units

            # ---------------- attention for one i-block ---------------------
            def emit_attention(n, fillers):
                """Emit the attention strips for block n=(b*IB+ib), draining
                filler units between strips, then the normalization tail."""
                b, ib = divmod(n, IB)
                i0 = b * T + ib * TB
                icols = slice(i0, i0 + TB)
                njt = 4 * (ib + 1)
                ot0 = psum.tile([128, TB], F32, tag="acc", bufs=4,
                                name=f"ot0_{n}")
                ot1 = psum.tile([128, TB], F32, tag="acc", bufs=4,
                                name=f"ot1_{n}")
                nf = len(fillers)
                fi = 0
                for jt in range(njt):
                    jg = b * NJT + jt
                    q = jt - (njt - 4)          # >=0 on diagonal strips
                    off = 128 * q if q > 0 else 0
                    w = TB - off
                    st = psum.tile([128, 2 * TB], F32, tag="st",
                                   name=f"st_{n}_{jt}")
                    for h in range(HL):
                        hs = slice(h * D, (h + 1) * D)
                        nc.tensor.matmul(
                            st[:, h * TB:(h + 1) * TB],
                            KT[hs, jg * 128:(jg + 1) * 128],
                            QT[hs, icols],
                            start=True, stop=True,
                        )
                    pt = work.tile([128, 2 * TB], BF16, tag="pt", bufs=4,
                                   name=f"pt_{n}_{jt}")
                    nc.scalar.activation(pt[:, :], st[:, :], AF.Exp,
                                         scale=0.125)
                    if q >= 0:
                        nc.vector.tensor_tensor(
                            pt[:, off:off + 128], pt[:, off:off + 128],
                            tri_sb[:, :], ALU.mult)
                        nc.vector.tensor_tensor(
                            pt[:, TB + off:TB + off + 128],
                            pt[:, TB + off:TB + off + 128],
                            tri_sb[:, :], ALU.mult)
                    g = jg * VW
                    nc.tensor.matmul(
                        ot0[0:65, off:TB],
                        V[:, g:g + 65],
                        pt[:, off:TB],
                        start=(jt == 0), stop=(jt == njt - 1),
                    )
                    nc.tensor.matmul(
                        ot1[0:65, off:TB],
                        V[:, g + 65:g + 130],
                        pt[:, TB + off:2 * TB],
                        start=(jt == 0), stop=(jt == njt - 1),
                    )
                    # drain filler units evenly across strips
                    want = (jt + 1) * nf // njt
                    while fi < want:
                        fillers[fi]()
                        fi += 1
                # ---- normalization tail (head1 first: its chain is longer;
                # sum-copies go to ACT to keep DVE free for the next block's
                # mask multiplies) ----
                s1 = work.tile([1, TB], F32, tag="s1", name=f"s1_{n}")
                nc.vector.tensor_copy(s1[:, :], ot1[64:65, :])
                r1 = work.tile([1, TB], F32, tag="r1", name=f"r1_{n}")
                nc.vector.reciprocal_approx_fast(r1[:, :], s1[:, :])
                bc1 = work.tile([64, TB], F32, tag="bc1", name=f"bc1_{n}")
                nc.gpsimd.partition_broadcast(bc1[:, :], r1[0:1, :])
                yn1 = work.tile([64, TB], BF16, tag="yn1", name=f"yn1_{n}")
                nc.vector.tensor_tensor(yn1[:, :], ot1[0:64, :],
                                        bc1[:, :], ALU.mult)
                if n == 7:
                    blk_state[7] = yn1
                else:
                    nc.sync.dma_start(ylocT[64:128, icols], yn1[:, :])
                s0 = work.tile([1, TB], F32, tag="s0", name=f"s0_{n}")
                nc.vector.tensor_copy(s0[:, :], ot0[64:65, :])
                r0 = work.tile([1, TB], F32, tag="r0", name=f"r0_{n}")
                nc.vector.reciprocal_approx_fast(r0[:, :], s0[:, :])
                bc0 = work.tile([64, TB], F32, tag="bc0", name=f"bc0_{n}")
                nc.gpsimd.partition_broadcast(bc0[:, :], r0[0:1, :])
                nc.vector.tensor_tensor(ylocT[0:64, icols], ot0[0:64, :],
                                        bc0[:, :], ALU.mult)
                if n == 7:
                    # keep the PE clock warm while the chain drains so the
                    # final output projection runs at full rate
                    wp2 = psum.tile([128, 512], F32, tag="st",
                                    name="warm_end")
                    for wi in range(12):
                        nc.tensor.matmul(wp2[:, :], warm_sb[:, 0:128],
                                         warm_sb[:, 0:512],
                                         start=(wi == 0), stop=(wi == 11))

            # ---------------- schedule ----------------
            load_x(1)
            for u in qkv_units(0):
                u()
            # out-proj of block m is interleaved into a LATER block's strip
            # loop; short blocks (njt=4) are skipped so their strips aren't
            # head-blocked by not-yet-ready out-proj matmuls.
            pending_op = []          # blocks whose outproj still needs a home
            for n in range(8):
                fillers = []
                if n + 1 < NTB:
                    fillers += qkv_units(n + 1)
                ib = n % IB
                if 4 * (ib + 1) >= 8:
                    for m in pending_op:
                        fillers += outproj_units(m)
                    pending_op = []
                emit_attention(n, fillers)
                pending_op.append(n)
            for m in pending_op:
                for u in outproj_units(m):
                    u()
    nc.compile()
    return nc


def _host_inputs(x, Wq, bq, Wk, bk, Wv, bv, Wo):
    """Build the 8 per-core input maps (host-side layout prep + sharding)."""
    import ml_dtypes
    bf16 = ml_dtypes.bfloat16
    xT = np.ascontiguousarray(x.reshape(BT, C).T.astype(bf16))  # [C, BT]
    # pack so each t-block load is one contiguous [128, NKT*TB] slab:
    # xP[p, tb*4096 + ct*512 + t] = xT[ct*128 + p, tb*512 + t]
    xP = np.ascontiguousarray(
        xT.reshape(NKT, 128, NTB, TB).transpose(1, 2, 0, 3)
        .reshape(128, NTB * NKT * TB))
    tri = np.triu(np.ones((128, 128), np.float32)).astype(bf16)  # [jl, il>=jl]
    ident = np.eye(128, dtype=bf16)

    def wtile(W, rows):
        # device layout: w_sb[p, k*128 + j] = W[rows][j, k*128 + p]
        wT = W[rows, :].T.astype(bf16)                # [C, CL]
        return np.ascontiguousarray(
            wT.reshape(NKT, 128, CL).transpose(1, 0, 2).reshape(128, NKT * CL))

    in_maps = []
    for core in range(NCORES):
        rows = slice(core * CL, (core + 1) * CL)
        in_maps.append({
            "xP": xP,
            "wqT": wtile(Wq, rows),
            "wkT": wtile(Wk, rows),
            "wvT": wtile(Wv, rows),
            "woT": np.ascontiguousarray(Wo[:, rows].T.astype(bf16)),
            "bq": np.ascontiguousarray(bq[rows].reshape(CL, 1).astype(np.float32)),
            "bk": np.ascontiguousarray(bk[rows].reshape(CL, 1).astype(np.float32)),
            "bv": np.ascontiguousarray(bv[rows].reshape(CL, 1).astype(np.float32)),
            "tri": tri,
            "ident": ident,
        })
    return in_maps


_NC_CACHE = None


def _get_nc():
    global _NC_CACHE
    if _NC_CACHE is None:
        _NC_CACHE = build_nc()
    return _NC_CACHE


def _run(inputs, trace=False):
    x = np.asarray(inputs["x"], np.float32)
    in_maps = _host_inputs(
        x,
        np.asarray(inputs["Wq"], np.float32), np.asarray(inputs["bq"], np.float32),
        np.asarray(inputs["Wk"], np.float32), np.asarray(inputs["bk"], np.float32),
        np.asarray(inputs["Wv"], np.float32), np.asarray(inputs["bv"], np.float32),
        np.asarray(inputs["Wo"], np.float32),
    )
    res = run_bass_kernel_spmd(_get_nc(), in_maps, list(range(NCORES)), trace=trace)
    # yP[n, p, cp*?]: [8 iblocks, 4 co-pairs packed] -> y[BT, C]
    yT = np.zeros((NTB * 4, 128, 1024), np.float64)
    for core in range(NCORES):
        yT += res.results[core]["yP"].astype(np.float64)
    # unpack: y[n*512 + t, co*128 + p] = yP[n*4 + cp, p, half*512 + t],
    # co = cp*2 + half
    y = np.zeros((BT, C), np.float64)
    yv = yT.reshape(NTB, 4, 128, 2, TB)        # [n, cp, p, half, t]
    for cp in range(4):
        for half in range(2):
            co = cp * 2 + half
            # [n, p, t] -> y[n*512+t, co*128+p]
            blk = yv[:, cp, :, half, :]        # [NTB, 128, TB]
            y[:, co * 128:(co + 1) * 128] += blk.transpose(0, 2, 1).reshape(
                BT, 128)
    y = y.astype(np.float32) + np.asarray(inputs["bo"], np.float32)
    return y.reshape(B, T, C), res


def kernel(**inputs) -> np.ndarray:
    out, _ = _run(inputs, trace=False)
    return out


def _install_profile_hook():
    """Register the axon NTFF profile hook (the agent image ships the ctypes
    shim in trn_agent_boot but lacks the antenv.axon_hooks module)."""
    import types

    if "antenv.axon_hooks" in sys.modules:
        return
    sys.path.insert(0, "/root/.axon_site")
    from trn_agent_boot.trn_boot import _ntff_profile_via_ctypes

    mod = types.ModuleType("antenv.axon_hooks")
    hook = _ntff_profile_via_ctypes("/opt/axon/libaxon_pjrt.so")
    mod.get_axon_ntff_profile_hook = lambda: hook
    mod.set_axon_ntff_profile_hook = lambda h: None
    sys.modules["antenv.axon_hooks"] = mod
    import antenv

    antenv.axon_hooks = mod
    from concourse import bass_utils as _bu

    _bu.upload_artifacts = lambda tmpdir: tmpdir  # keep artifacts local


def kernel_profiled(**inputs):
    """Returns (output, exec_time_ns) using the NTFF profile of core 0."""
    _install_profile_hook()
    out, res = _run(inputs, trace=True)
    return out, res.exec_time_ns


# revision 35
# speedup vs baseline: 1.0721x; 1.0266x over previous
"""Causal self-attention (B=2, T=2048, C=1024, H=16) on 8 TRN2 NeuronCores.

Megatron-style tensor parallelism over heads: each core computes 2 of the 16
heads (Wq/Wk/Wv column-sharded, Wo row-sharded) and produces a partial output
projection; the partials are summed on the host (the all-reduce).

Per-core device dataflow (everything kept transposed so the PE contraction dim
is always the partition dim, no on-device transposes of x needed):
  xP  [128, NTB*NKT*TB] host-packed so each t-block load is one big DMA
  QT/KT/VT = W_locT.T @ xP tiles  (bf16 matmuls, K-tiled over C)
  V tiles   = PE-transpose of VT; head0 stationary [V0|1] (65 cols),
              head1 stationary [0|1|0..|V1] (128 cols) so head1's attention
              output lands on partitions 64-127 directly
  S^T       = K_loc @ Q_loc^T per (batch, head, 128-j-strip, 512-i-block);
              the two heads are packed into PE row groups (contraction D=64,
              tile_position rows 0-63 / 64-127 -> they run concurrently).
              Diagonal strips are ragged: only the unmasked i-suffix is
              computed (moving width w = 512-128q).
  P^T       = exp(S^T / 8) on ACT (no max-subtraction needed: |S|<~3),
              triangular [128,128] mask multiply on the diagonal sub-block
  O^T|s     = [V|1].T @ P^T accumulated over strips (ones col gives sums)
  ylocT     = O^T * (1/s)  (reciprocal + GpSimd partition broadcast)
  yT_part   = Wo_locT.T @ ylocT -> packed DRAM
Host: y = (sum_cores yT_part) + bo, reshape to [B, T, C].

Schedule: PE warmup burst, QKV(0) prologue, then 8 attention phases
(b, ib); QKV(tb+1) and the previous block's output projection are emitted
as filler units interleaved between attention strips so the PE never
idles long enough for the HAM clock gate to re-throttle.
"""

import sys

if "/opt/trn_rl_repo" not in sys.path:
    sys.path.insert(0, "/opt/trn_rl_repo")

import numpy as np

import concourse.bass as bass
import concourse.tile as tile
from concourse import bacc
from concourse import mybir
from concourse.bass_utils import run_bass_kernel_spmd

F32 = mybir.dt.float32
BF16 = mybir.dt.bfloat16
AF = mybir.ActivationFunctionType
ALU = mybir.AluOpType

B, T, C, H = 2, 2048, 1024, 16
D = C // H          # 64
NCORES = 8
HL = H // NCORES    # 2 local heads
CL = C // NCORES    # 128 local channels
BT = B * T          # 4096
TB = 512            # t-block / i-block width
NTB = BT // TB      # 8
NKT = C // 128      # 8 contraction tiles for projections
IB = T // TB        # 4 i-blocks per batch
NJT = T // 128      # 16 j-tiles per batch
VW = 130            # V group width: [V0|1]=65 + [V1|1]=65
NJG = BT // 128     # 32 j-groups total


def build_nc() -> bass.Bass:
    nc = bacc.Bacc()

    x_d = nc.declare_dram_parameter("xP", [128, NTB * NKT * TB], BF16,
                                    isOutput=False)
    wqT_d = nc.declare_dram_parameter("wqT", [128, C], BF16, isOutput=False)
    wkT_d = nc.declare_dram_parameter("wkT", [128, C], BF16, isOutput=False)
    wvT_d = nc.declare_dram_parameter("wvT", [128, C], BF16, isOutput=False)
    woT_d = nc.declare_dram_parameter("woT", [CL, C], BF16, isOutput=False)
    bq_d = nc.declare_dram_parameter("bq", [CL, 1], F32, isOutput=False)
    bk_d = nc.declare_dram_parameter("bk", [CL, 1], F32, isOutput=False)
    bv_d = nc.declare_dram_parameter("bv", [CL, 1], F32, isOutput=False)
    tri_d = nc.declare_dram_parameter("tri", [128, 128], BF16, isOutput=False)
    id_d = nc.declare_dram_parameter("ident", [128, 128], BF16, isOutput=False)
    # output packed: [8 iblocks, 4 co-pairs, 128 p, 1024]
    y_d = nc.declare_dram_parameter("yP", [NTB * 4, 128, 1024], BF16,
                                    isOutput=True)

    with tile.TileContext(nc) as tc:
        with (
            tc.tile_pool(name="const", bufs=1) as const,
            tc.tile_pool(name="work", bufs=2) as work,
            tc.tile_pool(name="psum", bufs=2, space="PSUM") as psum,
        ):
            # ---------------- PE warmup (keep HAM busy during DMA head) -----
            warm_sb = const.tile([128, 512], BF16)
            nc.gpsimd.memset(warm_sb[:, :], 0.0)
            ones_sb = const.tile([1, 64], BF16)
            nc.gpsimd.memset(ones_sb[:, :], 1.0)
            wp = psum.tile([128, 256], F32, tag="acc", bufs=4, name="warm")
            for wi in range(28):
                nc.tensor.matmul(wp[:, :], warm_sb[:, 0:128],
                                 warm_sb[:, 128:384],
                                 start=(wi == 0), stop=(wi == 27))

            # ---------------- constants / persistent state ----------------
            # first x t-block load goes first so the prologue QKV can start
            wq_sb = const.tile([128, C], BF16)
            nc.sync.dma_start(wq_sb[:, :], wqT_d[:, :])
            xt0 = work.tile([128, NKT * TB], BF16, tag="xt", bufs=4,
                            name="xt_0")
            nc.sync.dma_start(xt0[:, 0:NKT * TB // 2], x_d[:, 0:NKT * TB // 2])
            nc.sync.dma_start(xt0[:, NKT * TB // 2:NKT * TB],
                              x_d[:, NKT * TB // 2:NKT * TB])
            wk_sb = const.tile([128, C], BF16)
            wv_sb = const.tile([128, C], BF16)
            nc.sync.dma_start(wk_sb[:, :], wkT_d[:, :])
            nc.sync.dma_start(wv_sb[:, :], wvT_d[:, :])
            bq_sb = const.tile([128, 1], F32)
            nc.sync.dma_start(bq_sb[:, :], bq_d[:, :])
            bk_sb = const.tile([128, 1], F32)
            nc.sync.dma_start(bk_sb[:, :], bk_d[:, :])
            bv_sb = const.tile([128, 1], F32)
            nc.sync.dma_start(bv_sb[:, :], bv_d[:, :])
            id_sb = const.tile([128, 128], BF16)
            nc.sync.dma_start(id_sb[:, :], id_d[:, :])
            tri_sb = const.tile([128, 128], BF16)
            nc.sync.dma_start(tri_sb[:, :], tri_d[:, :])
            wo_sb = const.tile([128, C], BF16)
            nc.sync.dma_start(wo_sb[:, :], woT_d[:, :])
            # head1 rows of woT re-based to partitions 0-63: lets the last
            # block's out-proj contract per head (no ylocT upper-half DMA)
            wo2_sb = const.tile([64, C], BF16)
            nc.sync.dma_start(wo2_sb[:, :], woT_d[64:128, :])

            QT = const.tile([128, BT], BF16)
            KT = const.tile([128, BT], BF16)
            ylocT = const.tile([128, BT], BF16)
            V = const.tile([128, NJG * VW], BF16)
            # zero head1 stationary cols 0..63, set the two ones columns
            v3 = V[:, :].rearrange("p (g c) -> p g c", c=VW)
            nc.gpsimd.memset(v3[:, :, 64:65], 1.0)
            nc.gpsimd.memset(v3[:, :, 129:130], 1.0)

            # ---------------- QKV projection for one t-block ----------------
            xt_tiles = {0: xt0}

            def load_x(tb):
                """Start the DMA for t-block tb's x slab (prefetched one
                phase ahead of the fillers that consume it)."""
                if tb in xt_tiles or tb >= NTB:
                    return
                xt = work.tile([128, NKT * TB], BF16, tag="xt", bufs=4,
                               name=f"xt_{tb}")
                xt_tiles[tb] = xt
                nc.sync.dma_start(
                    xt[:, :], x_d[:, tb * NKT * TB:(tb + 1) * NKT * TB])

            def qkv_units(tb):
                """Returns a list of closures; each emits a chunk of the
                QKV+V-transpose work for t-block tb."""
                tcols = slice(tb * TB, (tb + 1) * TB)
                xt = xt_tiles[tb]
                state = {}

                def u_load():
                    load_x(tb + 2)

                def mk_chain(which, w_sb, b_sb):
                    def u_mm_a():
                        ps = psum.tile([128, TB], F32, tag="acc", bufs=4,
                                       name=f"ps_{which}_{tb}")
                        state[which] = ps
                        for ct in range(4):
                            nc.tensor.matmul(
                                ps[:, :],
                                w_sb[:, ct * 128:(ct + 1) * 128],
                                xt[:, ct * TB:(ct + 1) * TB],
                                start=(ct == 0), stop=False,
                            )

                    def u_mm_b():
                        ps = state[which]
                        for ct in range(4, NKT):
                            nc.tensor.matmul(
                                ps[:, :],
                                w_sb[:, ct * 128:(ct + 1) * 128],
                                xt[:, ct * TB:(ct + 1) * TB],
                                start=False, stop=(ct == NKT - 1),
                            )

                    def u_evac():
                        ps = state[which]
                        if which == "q":
                            nc.vector.tensor_scalar_add(QT[:, tcols], ps[:, :],
                                                        b_sb[:, :])
                        elif which == "k":
                            nc.vector.tensor_scalar_add(KT[:, tcols], ps[:, :],
                                                        b_sb[:, :])
                        else:
                            vt = work.tile([128, TB], BF16, tag="vt", bufs=2,
                                           name=f"vt_{tb}")
                            state["vt"] = vt
                            nc.vector.tensor_scalar_add(vt[:, :], ps[:, :],
                                                        b_sb[:, :])
                    return [u_mm_a, u_mm_b, u_evac]

                def mk_vtrans(q):
                    def u_vt():
                        vt = state["vt"]
                        jg = tb * 4 + q
                        tp = psum.tile([128, 128], BF16, tag="acc", bufs=4,
                                       name=f"tp_{jg}")
                        nc.tensor.transpose(tp[:, :],
                                            vt[:, q * 128:(q + 1) * 128],
                                            id_sb[:, :])
                        off = jg * VW
                        nc.vector.tensor_copy(V[:, off:off + 64], tp[:, 0:64])
                        nc.vector.tensor_copy(V[:, off + 65:off + 129],
                                              tp[:, 64:128])
                    return u_vt

                units = [u_load]
                units += mk_chain("q", wq_sb, bq_sb)
                units += mk_chain("k", wk_sb, bk_sb)
                units += mk_chain("v", wv_sb, bv_sb)
                units += [mk_vtrans(q) for q in range(4)]
                return units

            # -------- output projection units for one finished i-block ------
            blk_state = {}

            def outproj_units(n):
                b, ib = divmod(n, IB)
                i0 = b * T + ib * TB
                icols = slice(i0, i0 + TB)
                state = {}
                units = []

                def mk_co(co):
                    def u_co():
                        yp = psum.tile([128, TB], F32, tag="acc", bufs=4,
                                       name=f"yp_{n}_{co}")
                        if n == 7:
                            # per-head contraction: head1 half comes straight
                            # from yn1 (no ylocT upper-half assembly DMA)
                            nc.tensor.matmul(
                                yp[:, :],
                                wo_sb[0:64, co * 128:(co + 1) * 128],
                                ylocT[0:64, icols],
                                start=True, stop=False,
                            )
                            nc.tensor.matmul(
                                yp[:, :],
                                wo2_sb[:, co * 128:(co + 1) * 128],
                                blk_state[7][:, :],
                                start=False, stop=True,
                            )
                        else:
                            nc.tensor.matmul(
                                yp[:, :],
                                wo_sb[:, co * 128:(co + 1) * 128],
                                ylocT[:, icols],
                                start=True, stop=True,
                            )
                        cp, half = divmod(co, 2)
                        if half == 0:
                            yo = work.tile([128, 1024], BF16, tag="yo", bufs=3,
                                           name=f"yo_{n}_{cp}")
                            state[cp] = yo
                        else:
                            yo = state[cp]
                        hs = slice(half * TB, (half + 1) * TB)
                        if n == 7 and co >= 5:
                            nc.scalar.copy(yo[:, hs], yp[:, :])
                        else:
                            nc.vector.tensor_copy(yo[:, hs], yp[:, :])
                        if half == 1:
                            nc.sync.dma_start(y_d[n * 4 + cp, :, :], yo[:, :])
                    return u_co

                for co in range(8):
                    units.append(mk_co(co))
                return units

            # ---------------- attention for one i-block ---------------------
            def emit_attention(n, fillers):
                """Emit the attention strips for block n=(b*IB+ib), draining
                filler units between strips, then the normalization tail."""
                b, ib = divmod(n, IB)
                i0 = b * T + ib * TB
                icols = slice(i0, i0 + TB)
                njt = 4 * (ib + 1)
                ot0 = psum.tile([128, TB], F32, tag="acc", bufs=4,
                                name=f"ot0_{n}")
                ot1 = psum.tile([128, TB], F32, tag="acc", bufs=4,
                                name=f"ot1_{n}")
                nf = len(fillers)
                fi = 0
                if njt <= 4:
                    # short block: run all fillers first — they only need the
                    # 'acc' slots that this block's ot pair would otherwise
                    # pin across the whole (ACT-paced) phase
                    while fi < nf:
                        fillers[fi]()
                        fi += 1
                for jt in range(njt):
                    jg = b * NJT + jt
                    q = jt - (njt - 4)          # >=0 on diagonal strips
                    off = 128 * q if q > 0 else 0
                    w = TB - off
                    st = psum.tile([128, 2 * TB], F32, tag="st",
                                   name=f"st_{n}_{jt}")
                    for h in range(HL):
                        hs = slice(h * D, (h + 1) * D)
                        nc.tensor.matmul(
                            st[:, h * TB:(h + 1) * TB],
                            KT[hs, jg * 128:(jg + 1) * 128],
                            QT[hs, icols],
                            start=True, stop=True,
                        )
                    pt = work.tile([128, 2 * TB], BF16, tag="pt", bufs=4,
                                   name=f"pt_{n}_{jt}")
                    nc.scalar.activation(pt[:, :], st[:, :], AF.Exp,
                                         scale=0.125)
                    if q >= 0:
                        nc.vector.tensor_tensor(
                            pt[:, off:off + 128], pt[:, off:off + 128],
                            tri_sb[:, :], ALU.mult)
                        nc.vector.tensor_tensor(
                            pt[:, TB + off:TB + off + 128],
                            pt[:, TB + off:TB + off + 128],
                            tri_sb[:, :], ALU.mult)
                    g = jg * VW
                    nc.tensor.matmul(
                        ot0[0:65, off:TB],
                        V[:, g:g + 65],
                        pt[:, off:TB],
                        start=(jt == 0), stop=(jt == njt - 1),
                    )
                    nc.tensor.matmul(
                        ot1[0:65, off:TB],
                        V[:, g + 65:g + 130],
                        pt[:, TB + off:2 * TB],
                        start=(jt == 0), stop=(jt == njt - 1),
                    )
                    # drain filler units evenly across strips
                    want = (jt + 1) * nf // njt
                    while fi < want:
                        fillers[fi]()
                        fi += 1
                # ---- normalization tail (head1 first: its chain is longer;
                # sum-copies go to ACT to keep DVE free for the next block's
                # mask multiplies) ----
                s1 = work.tile([1, TB], F32, tag="s1", name=f"s1_{n}")
                nc.vector.tensor_copy(s1[:, :], ot1[64:65, :])
                r1 = work.tile([1, TB], F32, tag="r1", name=f"r1_{n}")
                nc.vector.reciprocal_approx_fast(r1[:, :], s1[:, :])
                bc1 = work.tile([64, TB], F32, tag="bc1", name=f"bc1_{n}")
                nc.gpsimd.partition_broadcast(bc1[:, :], r1[0:1, :])
                yn1 = work.tile([64, TB], BF16, tag="yn1", name=f"yn1_{n}")
                nc.vector.tensor_tensor(yn1[:, :], ot1[0:64, :],
                                        bc1[:, :], ALU.mult)
                if n == 7:
                    blk_state[7] = yn1
                else:
                    nc.sync.dma_start(ylocT[64:128, icols], yn1[:, :])
                s0 = work.tile([1, TB], F32, tag="s0", name=f"s0_{n}")
                nc.vector.tensor_copy(s0[:, :], ot0[64:65, :])
                r0 = work.tile([1, TB], F32, tag="r0", name=f"r0_{n}")
                nc.vector.reciprocal_approx_fast(r0[:, :], s0[:, :])
                bc0 = work.tile([64, TB], F32, tag="bc0", name=f"bc0_{n}")
                nc.gpsimd.partition_broadcast(bc0[:, :], r0[0:1, :])
                nc.vector.tensor_tensor(ylocT[0:64, icols], ot0[0:64, :],
                                        bc0[:, :], ALU.mult)
                if n == 7:
                    # keep the PE clock warm while the chain drains so the
                    # final output projection runs at full rate
                    wp2 = psum.tile([128, 512], F32, tag="st",
                                    name="warm_end")
                    for wi in range(12):
                        nc.tensor.matmul(wp2[:, :], warm_sb[:, 0:128],
                                         warm_sb[:, 0:512],
                                         start=(wi == 0), stop=(wi == 11))

            # ---------------- schedule ----------------
            load_x(1)
            for u in qkv_units(0):
                u()
            # out-proj of block m is interleaved into a LATER block's strip
            # loop; short blocks (njt=4) are skipped so their strips aren't
            # head-blocked by not-yet-ready out-proj matmuls.
            pending_op = []          # blocks whose outproj still needs a home
            for n in range(8):
                fillers = []
                if n + 1 < NTB:
                    fillers += qkv_units(n + 1)
                ib = n % IB
                if 4 * (ib + 1) >= 8:
                    for m in pending_op:
                        fillers += outproj_units(m)
                    pending_op = []
                emit_attention(n, fillers)
                pending_op.append(n)
            for m in pending_op:
                for u in outproj_units(m):
                    u()
    nc.compile()
    return nc


def _host_inputs(x, Wq, bq, Wk, bk, Wv, bv, Wo):
    """Build the 8 per-core input maps (host-side layout prep + sharding)."""
    import ml_dtypes
    bf16 = ml_dtypes.bfloat16
    xT = np.ascontiguousarray(x.reshape(BT, C).T.astype(bf16))  # [C, BT]
    # pack so each t-block load is one contiguous [128, NKT*TB] slab:
    # xP[p, tb*4096 + ct*512 + t] = xT[ct*128 + p, tb*512 + t]
    xP = np.ascontiguousarray(
        xT.reshape(NKT, 128, NTB, TB).transpose(1, 2, 0, 3)
        .reshape(128, NTB * NKT * TB))
    tri = np.triu(np.ones((128, 128), np.float32)).astype(bf16)  # [jl, il>=jl]
    ident = np.eye(128, dtype=bf16)

    def wtile(W, rows):
        # device layout: w_sb[p, k*128 + j] = W[rows][j, k*128 + p]
        wT = W[rows, :].T.astype(bf16)                # [C, CL]
        return np.ascontiguousarray(
            wT.reshape(NKT, 128, CL).transpose(1, 0, 2).reshape(128, NKT * CL))

    in_maps = []
    for core in range(NCORES):
        rows = slice(core * CL, (core + 1) * CL)
        in_maps.append({
            "xP": xP,
            "wqT": wtile(Wq, rows),
            "wkT": wtile(Wk, rows),
            "wvT": wtile(Wv, rows),
            "woT": np.ascontiguousarray(Wo[:, rows].T.astype(bf16)),
            "bq": np.ascontiguousarray(bq[rows].reshape(CL, 1).astype(np.float32)),
            "bk": np.ascontiguousarray(bk[rows].reshape(CL, 1).astype(np.float32)),
            "bv": np.ascontiguousarray(bv[rows].reshape(CL, 1).astype(np.float32)),
            "tri": tri,
            "ident": ident,
        })
    return in_maps


_NC_CACHE = None


def _get_nc():
    global _NC_CACHE
    if _NC_CACHE is None:
        _NC_CACHE = build_nc()
    return _NC_CACHE


def _run(inputs, trace=False):
    x = np.asarray(inputs["x"], np.float32)
    in_maps = _host_inputs(
        x,
        np.asarray(inputs["Wq"], np.float32), np.asarray(inputs["bq"], np.float32),
        np.asarray(inputs["Wk"], np.float32), np.asarray(inputs["bk"], np.float32),
        np.asarray(inputs["Wv"], np.float32), np.asarray(inputs["bv"], np.float32),
        np.asarray(inputs["Wo"], np.float32),
    )
    res = run_bass_kernel_spmd(_get_nc(), in_maps, list(range(NCORES)), trace=trace)
    # yP[n, p, cp*?]: [8 iblocks, 4 co-pairs packed] -> y[BT, C]
    yT = np.zeros((NTB * 4, 128, 1024), np.float64)
    for core in range(NCORES):
        yT += res.results[core]["yP"].astype(np.float64)
    # unpack: y[n*512 + t, co*128 + p] = yP[n*4 + cp, p, half*512 + t],
    # co = cp*2 + half
    y = np.zeros((BT, C), np.float64)
    yv = yT.reshape(NTB, 4, 128, 2, TB)        # [n, cp, p, half, t]
    for cp in range(4):
        for half in range(2):
            co = cp * 2 + half
            # [n, p, t] -> y[n*512+t, co*128+p]
            blk = yv[:, cp, :, half, :]        # [NTB, 128, TB]
            y[:, co * 128:(co + 1) * 128] += blk.transpose(0, 2, 1).reshape(
                BT, 128)
    y = y.astype(np.float32) + np.asarray(inputs["bo"], np.float32)
    return y.reshape(B, T, C), res


def kernel(**inputs) -> np.ndarray:
    out, _ = _run(inputs, trace=False)
    return out


def _install_profile_hook():
    """Register the axon NTFF profile hook (the agent image ships the ctypes
    shim in trn_agent_boot but lacks the antenv.axon_hooks module)."""
    import types

    if "antenv.axon_hooks" in sys.modules:
        return
    sys.path.insert(0, "/root/.axon_site")
    from trn_agent_boot.trn_boot import _ntff_profile_via_ctypes

    mod = types.ModuleType("antenv.axon_hooks")
    hook = _ntff_profile_via_ctypes("/opt/axon/libaxon_pjrt.so")
    mod.get_axon_ntff_profile_hook = lambda: hook
    mod.set_axon_ntff_profile_hook = lambda h: None
    sys.modules["antenv.axon_hooks"] = mod
    import antenv

    antenv.axon_hooks = mod
    from concourse import bass_utils as _bu

    _bu.upload_artifacts = lambda tmpdir: tmpdir  # keep artifacts local


def kernel_profiled(**inputs):
    """Returns (output, exec_time_ns) using the NTFF profile of core 0."""
    _install_profile_hook()
    out, res = _run(inputs, trace=True)
    return out, res.exec_time_ns


# revision 37
# speedup vs baseline: 1.0726x; 1.0004x over previous
"""Causal self-attention (B=2, T=2048, C=1024, H=16) on 8 TRN2 NeuronCores.

Megatron-style tensor parallelism over heads: each core computes 2 of the 16
heads (Wq/Wk/Wv column-sharded, Wo row-sharded) and produces a partial output
projection; the partials are summed on the host (the all-reduce).

Per-core device dataflow (everything kept transposed so the PE contraction dim
is always the partition dim, no on-device transposes of x needed):
  xP  [128, NTB*NKT*TB] host-packed so each t-block load is one big DMA
  QT/KT/VT = W_locT.T @ xP tiles  (bf16 matmuls, K-tiled over C)
  V tiles   = PE-transpose of VT; head0 stationary [V0|1] (65 cols),
              head1 stationary [0|1|0..|V1] (128 cols) so head1's attention
              output lands on partitions 64-127 directly
  S^T       = K_loc @ Q_loc^T per (batch, head, 128-j-strip, 512-i-block);
              the two heads are packed into PE row groups (contraction D=64,
              tile_position rows 0-63 / 64-127 -> they run concurrently).
              Diagonal strips are ragged: only the unmasked i-suffix is
              computed (moving width w = 512-128q).
  P^T       = exp(S^T / 8) on ACT (no max-subtraction needed: |S|<~3),
              triangular [128,128] mask multiply on the diagonal sub-block
  O^T|s     = [V|1].T @ P^T accumulated over strips (ones col gives sums)
  ylocT     = O^T * (1/s)  (reciprocal + GpSimd partition broadcast)
  yT_part   = Wo_locT.T @ ylocT -> packed DRAM
Host: y = (sum_cores yT_part) + bo, reshape to [B, T, C].

Schedule: PE warmup burst, QKV(0) prologue, then 8 attention phases
(b, ib); QKV(tb+1) and the previous block's output projection are emitted
as filler units interleaved between attention strips so the PE never
idles long enough for the HAM clock gate to re-throttle.
"""

import sys

if "/opt/trn_rl_repo" not in sys.path:
    sys.path.insert(0, "/opt/trn_rl_repo")

import numpy as np

import concourse.bass as bass
import concourse.tile as tile
from concourse import bacc
from concourse import mybir
from concourse.bass_utils import run_bass_kernel_spmd

F32 = mybir.dt.float32
BF16 = mybir.dt.bfloat16
AF = mybir.ActivationFunctionType
ALU = mybir.AluOpType

B, T, C, H = 2, 2048, 1024, 16
D = C // H          # 64
NCORES = 8
HL = H // NCORES    # 2 local heads
CL = C // NCORES    # 128 local channels
BT = B * T          # 4096
TB = 512            # t-block / i-block width
NTB = BT // TB      # 8
NKT = C // 128      # 8 contraction tiles for projections
IB = T // TB        # 4 i-blocks per batch
NJT = T // 128      # 16 j-tiles per batch
VW = 130            # V group width: [V0|1]=65 + [V1|1]=65
NJG = BT // 128     # 32 j-groups total


def build_nc() -> bass.Bass:
    nc = bacc.Bacc()

    x_d = nc.declare_dram_parameter("xP", [128, NTB * NKT * TB], BF16,
                                    isOutput=False)
    wqT_d = nc.declare_dram_parameter("wqT", [128, C], BF16, isOutput=False)
    wkT_d = nc.declare_dram_parameter("wkT", [128, C], BF16, isOutput=False)
    wvT_d = nc.declare_dram_parameter("wvT", [128, C], BF16, isOutput=False)
    woT_d = nc.declare_dram_parameter("woT", [CL, C], BF16, isOutput=False)
    bq_d = nc.declare_dram_parameter("bq", [CL, 1], F32, isOutput=False)
    bk_d = nc.declare_dram_parameter("bk", [CL, 1], F32, isOutput=False)
    bv_d = nc.declare_dram_parameter("bv", [CL, 1], F32, isOutput=False)
    tri_d = nc.declare_dram_parameter("tri", [128, 256], BF16, isOutput=False)
    id_d = nc.declare_dram_parameter("ident", [128, 128], BF16, isOutput=False)
    # output packed: [8 iblocks, 4 co-pairs, 128 p, 1024]
    y_d = nc.declare_dram_parameter("yP", [NTB * 4, 128, 1024], BF16,
                                    isOutput=True)

    with tile.TileContext(nc) as tc:
        with (
            tc.tile_pool(name="const", bufs=1) as const,
            tc.tile_pool(name="work", bufs=2) as work,
            tc.tile_pool(name="psum", bufs=2, space="PSUM") as psum,
        ):
            # ---------------- PE warmup (keep HAM busy during DMA head) -----
            warm_sb = const.tile([128, 512], BF16)
            nc.gpsimd.memset(warm_sb[:, :], 0.0)
            ones_sb = const.tile([1, 64], BF16)
            nc.gpsimd.memset(ones_sb[:, :], 1.0)
            wp = psum.tile([128, 256], F32, tag="acc", bufs=4, name="warm")
            for wi in range(28):
                nc.tensor.matmul(wp[:, :], warm_sb[:, 0:128],
                                 warm_sb[:, 128:384],
                                 start=(wi == 0), stop=(wi == 27))

            # ---------------- constants / persistent state ----------------
            # first x t-block load goes first so the prologue QKV can start
            wq_sb = const.tile([128, C], BF16)
            nc.sync.dma_start(wq_sb[:, :], wqT_d[:, :])
            xt0 = work.tile([128, NKT * TB], BF16, tag="xt", bufs=4,
                            name="xt_0")
            nc.sync.dma_start(xt0[:, 0:NKT * TB // 2], x_d[:, 0:NKT * TB // 2])
            nc.sync.dma_start(xt0[:, NKT * TB // 2:NKT * TB],
                              x_d[:, NKT * TB // 2:NKT * TB])
            wk_sb = const.tile([128, C], BF16)
            wv_sb = const.tile([128, C], BF16)
            nc.sync.dma_start(wk_sb[:, :], wkT_d[:, :])
            nc.sync.dma_start(wv_sb[:, :], wvT_d[:, :])
            bq_sb = const.tile([128, 1], F32)
            nc.sync.dma_start(bq_sb[:, :], bq_d[:, :])
            bk_sb = const.tile([128, 1], F32)
            nc.sync.dma_start(bk_sb[:, :], bk_d[:, :])
            bv_sb = const.tile([128, 1], F32)
            nc.sync.dma_start(bv_sb[:, :], bv_d[:, :])
            id_sb = const.tile([128, 128], BF16)
            nc.sync.dma_start(id_sb[:, :], id_d[:, :])
            tri_sb = const.tile([128, 256], BF16)
            nc.sync.dma_start(tri_sb[:, :], tri_d[:, :])
            wo_sb = const.tile([128, C], BF16)
            nc.sync.dma_start(wo_sb[:, :], woT_d[:, :])
            # head1 rows of woT re-based to partitions 0-63: lets the last
            # block's out-proj contract per head (no ylocT upper-half DMA)
            wo2_sb = const.tile([64, C], BF16)
            nc.sync.dma_start(wo2_sb[:, :], woT_d[64:128, :])

            QT = const.tile([128, BT], BF16)
            KT = const.tile([128, BT], BF16)
            ylocT = const.tile([128, BT], BF16)
            V = const.tile([128, NJG * VW], BF16)
            # zero head1 stationary cols 0..63, set the two ones columns
            v3 = V[:, :].rearrange("p (g c) -> p g c", c=VW)
            nc.gpsimd.memset(v3[:, :, 64:65], 1.0)
            nc.gpsimd.memset(v3[:, :, 129:130], 1.0)

            # ---------------- QKV projection for one t-block ----------------
            xt_tiles = {0: xt0}

            def load_x(tb):
                """Start the DMA for t-block tb's x slab (prefetched one
                phase ahead of the fillers that consume it)."""
                if tb in xt_tiles or tb >= NTB:
                    return
                xt = work.tile([128, NKT * TB], BF16, tag="xt", bufs=4,
                               name=f"xt_{tb}")
                xt_tiles[tb] = xt
                nc.sync.dma_start(
                    xt[:, :], x_d[:, tb * NKT * TB:(tb + 1) * NKT * TB])

            def qkv_units(tb):
                """Returns a list of closures; each emits a chunk of the
                QKV+V-transpose work for t-block tb."""
                tcols = slice(tb * TB, (tb + 1) * TB)
                xt = xt_tiles[tb]
                state = {}

                def u_load():
                    load_x(tb + 2)

                def mk_chain(which, w_sb, b_sb):
                    def u_mm_a():
                        ps = psum.tile([128, TB], F32, tag="acc", bufs=4,
                                       name=f"ps_{which}_{tb}")
                        state[which] = ps
                        for ct in range(4):
                            nc.tensor.matmul(
                                ps[:, :],
                                w_sb[:, ct * 128:(ct + 1) * 128],
                                xt[:, ct * TB:(ct + 1) * TB],
                                start=(ct == 0), stop=False,
                            )

                    def u_mm_b():
                        ps = state[which]
                        for ct in range(4, NKT):
                            nc.tensor.matmul(
                                ps[:, :],
                                w_sb[:, ct * 128:(ct + 1) * 128],
                                xt[:, ct * TB:(ct + 1) * TB],
                                start=False, stop=(ct == NKT - 1),
                            )

                    def u_evac():
                        ps = state[which]
                        if which == "q":
                            nc.vector.tensor_scalar_add(QT[:, tcols], ps[:, :],
                                                        b_sb[:, :])
                        elif which == "k":
                            nc.vector.tensor_scalar_add(KT[:, tcols], ps[:, :],
                                                        b_sb[:, :])
                        else:
                            vt = work.tile([128, TB], BF16, tag="vt", bufs=2,
                                           name=f"vt_{tb}")
                            state["vt"] = vt
                            nc.vector.tensor_scalar_add(vt[:, :], ps[:, :],
                                                        b_sb[:, :])
                    return [u_mm_a, u_mm_b, u_evac]

                def mk_vtrans(q):
                    def u_vt():
                        vt = state["vt"]
                        jg = tb * 4 + q
                        tp = psum.tile([128, 128], BF16, tag="acc", bufs=4,
                                       name=f"tp_{jg}")
                        nc.tensor.transpose(tp[:, :],
                                            vt[:, q * 128:(q + 1) * 128],
                                            id_sb[:, :])
                        off = jg * VW
                        nc.vector.tensor_copy(V[:, off:off + 64], tp[:, 0:64])
                        nc.vector.tensor_copy(V[:, off + 65:off + 129],
                                              tp[:, 64:128])
                    return u_vt

                units = [u_load]
                units += mk_chain("q", wq_sb, bq_sb)
                units += mk_chain("k", wk_sb, bk_sb)
                units += mk_chain("v", wv_sb, bv_sb)
                units += [mk_vtrans(q) for q in range(4)]
                return units

            # -------- output projection units for one finished i-block ------
            blk_state = {}

            def outproj_units(n):
                b, ib = divmod(n, IB)
                i0 = b * T + ib * TB
                icols = slice(i0, i0 + TB)
                state = {}
                units = []

                def mk_co(co):
                    def u_co():
                        yp = psum.tile([128, TB], F32, tag="acc", bufs=4,
                                       name=f"yp_{n}_{co}")
                        if n == 7:
                            # per-head contraction: head1 half comes straight
                            # from yn1 (no ylocT upper-half assembly DMA)
                            nc.tensor.matmul(
                                yp[:, :],
                                wo_sb[0:64, co * 128:(co + 1) * 128],
                                ylocT[0:64, icols],
                                start=True, stop=False,
                            )
                            nc.tensor.matmul(
                                yp[:, :],
                                wo2_sb[:, co * 128:(co + 1) * 128],
                                blk_state[7][:, :],
                                start=False, stop=True,
                            )
                        else:
                            nc.tensor.matmul(
                                yp[:, :],
                                wo_sb[:, co * 128:(co + 1) * 128],
                                ylocT[:, icols],
                                start=True, stop=True,
                            )
                        cp, half = divmod(co, 2)
                        if half == 0:
                            yo = work.tile([128, 1024], BF16, tag="yo", bufs=3,
                                           name=f"yo_{n}_{cp}")
                            state[cp] = yo
                        else:
                            yo = state[cp]
                        hs = slice(half * TB, (half + 1) * TB)
                        if (n == 7 and co >= 5) or (n <= 3 and co % 2 == 0):
                            nc.scalar.copy(yo[:, hs], yp[:, :])
                        else:
                            nc.vector.tensor_copy(yo[:, hs], yp[:, :])
                        if half == 1:
                            nc.sync.dma_start(y_d[n * 4 + cp, :, :], yo[:, :])
                    return u_co

                for co in range(8):
                    units.append(mk_co(co))
                return units

            # ---------------- attention for one i-block ---------------------
            def emit_attention(n, fillers):
                """Emit the attention strips for block n=(b*IB+ib), draining
                filler units between strips, then the normalization tail."""
                b, ib = divmod(n, IB)
                i0 = b * T + ib * TB
                icols = slice(i0, i0 + TB)
                njt = 4 * (ib + 1)
                ot0 = psum.tile([128, TB], F32, tag="acc", bufs=4,
                                name=f"ot0_{n}")
                ot1 = psum.tile([128, TB], F32, tag="acc", bufs=4,
                                name=f"ot1_{n}")
                nf = len(fillers)
                fi = 0
                if njt <= 4:
                    # short block: run all fillers first — they only need the
                    # 'acc' slots that this block's ot pair would otherwise
                    # pin across the whole (ACT-paced) phase
                    while fi < nf:
                        fillers[fi]()
                        fi += 1
                for jt in range(njt):
                    jg = b * NJT + jt
                    q = jt - (njt - 4)          # >=0 on diagonal strips
                    off = 128 * q if q > 0 else 0
                    w = TB - off
                    st = psum.tile([128, 2 * TB], F32, tag="st",
                                   name=f"st_{n}_{jt}")
                    for h in range(HL):
                        hs = slice(h * D, (h + 1) * D)
                        nc.tensor.matmul(
                            st[:, h * TB:(h + 1) * TB],
                            KT[hs, jg * 128:(jg + 1) * 128],
                            QT[hs, icols],
                            start=True, stop=True,
                        )
                    pt = work.tile([128, 2 * TB], BF16, tag="pt", bufs=4,
                                   name=f"pt_{n}_{jt}")
                    nc.scalar.activation(pt[:, :], st[:, :], AF.Exp,
                                         scale=0.125)
                    if q >= 0:
                        ptv = pt[:, :].rearrange("p (h w) -> p h w", h=2)
                        ptm = ptv[:, :, off:off + 128]
                        triv = tri_sb[:, :].rearrange("p (h c) -> p h c", h=2)
                        nc.vector.tensor_tensor(ptm, ptm, triv, ALU.mult)
                    g = jg * VW
                    nc.tensor.matmul(
                        ot0[0:65, off:TB],
                        V[:, g:g + 65],
                        pt[:, off:TB],
                        start=(jt == 0), stop=(jt == njt - 1),
                    )
                    nc.tensor.matmul(
                        ot1[0:65, off:TB],
                        V[:, g + 65:g + 130],
                        pt[:, TB + off:2 * TB],
                        start=(jt == 0), stop=(jt == njt - 1),
                    )
                    # drain filler units evenly across strips
                    want = (jt + 1) * nf // njt
                    while fi < want:
                        fillers[fi]()
                        fi += 1
                # ---- normalization tail (head1 first: its chain is longer;
                # sum-copies go to ACT to keep DVE free for the next block's
                # mask multiplies) ----
                s1 = work.tile([1, TB], F32, tag="s1", name=f"s1_{n}")
                nc.vector.tensor_copy(s1[:, :], ot1[64:65, :])
                r1 = work.tile([1, TB], F32, tag="r1", name=f"r1_{n}")
                nc.vector.reciprocal_approx_fast(r1[:, :], s1[:, :])
                bc1 = work.tile([64, TB], F32, tag="bc1", name=f"bc1_{n}")
                nc.gpsimd.partition_broadcast(bc1[:, :], r1[0:1, :])
                yn1 = work.tile([64, TB], BF16, tag="yn1", name=f"yn1_{n}")
                nc.vector.tensor_tensor(yn1[:, :], ot1[0:64, :],
                                        bc1[:, :], ALU.mult)
                if n == 7:
                    blk_state[7] = yn1
                else:
                    nc.sync.dma_start(ylocT[64:128, icols], yn1[:, :])
                s0 = work.tile([1, TB], F32, tag="s0", name=f"s0_{n}")
                nc.vector.tensor_copy(s0[:, :], ot0[64:65, :])
                r0 = work.tile([1, TB], F32, tag="r0", name=f"r0_{n}")
                nc.vector.reciprocal_approx_fast(r0[:, :], s0[:, :])
                bc0 = work.tile([64, TB], F32, tag="bc0", name=f"bc0_{n}")
                nc.gpsimd.partition_broadcast(bc0[:, :], r0[0:1, :])
                nc.vector.tensor_tensor(ylocT[0:64, icols], ot0[0:64, :],
                                        bc0[:, :], ALU.mult)
                if n == 7:
                    # keep the PE clock warm while the chain drains so the
                    # final output projection runs at full rate
                    wp2 = psum.tile([128, 512], F32, tag="st",
                                    name="warm_end")
                    for wi in range(18):
                        nc.tensor.matmul(wp2[:, :], warm_sb[:, 0:128],
                                         warm_sb[:, 0:512],
                                         start=(wi == 0), stop=(wi == 17))

            # ---------------- schedule ----------------
            load_x(1)
            for u in qkv_units(0):
                u()
            # out-proj of block m is interleaved into a LATER block's strip
            # loop; short blocks (njt=4) are skipped so their strips aren't
            # head-blocked by not-yet-ready out-proj matmuls.
            pending_op = []          # blocks whose outproj still needs a home
            for n in range(8):
                fillers = []
                if n + 1 < NTB:
                    fillers += qkv_units(n + 1)
                ib = n % IB
                if 4 * (ib + 1) >= 8:
                    for m in pending_op:
                        fillers += outproj_units(m)
                    pending_op = []
                emit_attention(n, fillers)
                pending_op.append(n)
            for m in pending_op:
                for u in outproj_units(m):
                    u()
    nc.compile()
    return nc


def _host_inputs(x, Wq, bq, Wk, bk, Wv, bv, Wo):
    """Build the 8 per-core input maps (host-side layout prep + sharding)."""
    import ml_dtypes
    bf16 = ml_dtypes.bfloat16
    xT = np.ascontiguousarray(x.reshape(BT, C).T.astype(bf16))  # [C, BT]
    # pack so each t-block load is one contiguous [128, NKT*TB] slab:
    # xP[p, tb*4096 + ct*512 + t] = xT[ct*128 + p, tb*512 + t]
    xP = np.ascontiguousarray(
        xT.reshape(NKT, 128, NTB, TB).transpose(1, 2, 0, 3)
        .reshape(128, NTB * NKT * TB))
    tri1 = np.triu(np.ones((128, 128), np.float32)).astype(bf16)  # [jl, il>=jl]
    tri = np.ascontiguousarray(np.concatenate([tri1, tri1], axis=1))
    ident = np.eye(128, dtype=bf16)

    def wtile(W, rows):
        # device layout: w_sb[p, k*128 + j] = W[rows][j, k*128 + p]
        wT = W[rows, :].T.astype(bf16)                # [C, CL]
        return np.ascontiguousarray(
            wT.reshape(NKT, 128, CL).transpose(1, 0, 2).reshape(128, NKT * CL))

    in_maps = []
    for core in range(NCORES):
        rows = slice(core * CL, (core + 1) * CL)
        in_maps.append({
            "xP": xP,
            "wqT": wtile(Wq, rows),
            "wkT": wtile(Wk, rows),
            "wvT": wtile(Wv, rows),
            "woT": np.ascontiguousarray(Wo[:, rows].T.astype(bf16)),
            "bq": np.ascontiguousarray(bq[rows].reshape(CL, 1).astype(np.float32)),
            "bk": np.ascontiguousarray(bk[rows].reshape(CL, 1).astype(np.float32)),
            "bv": np.ascontiguousarray(bv[rows].reshape(CL, 1).astype(np.float32)),
            "tri": tri,
            "ident": ident,
        })
    return in_maps


_NC_CACHE = None


def _get_nc():
    global _NC_CACHE
    if _NC_CACHE is None:
        _NC_CACHE = build_nc()
    return _NC_CACHE


def _run(inputs, trace=False):
    x = np.asarray(inputs["x"], np.float32)
    in_maps = _host_inputs(
        x,
        np.asarray(inputs["Wq"], np.float32), np.asarray(inputs["bq"], np.float32),
        np.asarray(inputs["Wk"], np.float32), np.asarray(inputs["bk"], np.float32),
        np.asarray(inputs["Wv"], np.float32), np.asarray(inputs["bv"], np.float32),
        np.asarray(inputs["Wo"], np.float32),
    )
    res = run_bass_kernel_spmd(_get_nc(), in_maps, list(range(NCORES)), trace=trace)
    # yP[n, p, cp*?]: [8 iblocks, 4 co-pairs packed] -> y[BT, C]
    yT = np.zeros((NTB * 4, 128, 1024), np.float64)
    for core in range(NCORES):
        yT += res.results[core]["yP"].astype(np.float64)
    # unpack: y[n*512 + t, co*128 + p] = yP[n*4 + cp, p, half*512 + t],
    # co = cp*2 + half
    y = np.zeros((BT, C), np.float64)
    yv = yT.reshape(NTB, 4, 128, 2, TB)        # [n, cp, p, half, t]
    for cp in range(4):
        for half in range(2):
            co = cp * 2 + half
            # [n, p, t] -> y[n*512+t, co*128+p]
            blk = yv[:, cp, :, half, :]        # [NTB, 128, TB]
            y[:, co * 128:(co + 1) * 128] += blk.transpose(0, 2, 1).reshape(
                BT, 128)
    y = y.astype(np.float32) + np.asarray(inputs["bo"], np.float32)
    return y.reshape(B, T, C), res


def kernel(**inputs) -> np.ndarray:
    out, _ = _run(inputs, trace=False)
    return out


def _install_profile_hook():
    """Register the axon NTFF profile hook (the agent image ships the ctypes
    shim in trn_agent_boot but lacks the antenv.axon_hooks module)."""
    import types

    if "antenv.axon_hooks" in sys.modules:
        return
    sys.path.insert(0, "/root/.axon_site")
    from trn_agent_boot.trn_boot import _ntff_profile_via_ctypes

    mod = types.ModuleType("antenv.axon_hooks")
    hook = _ntff_profile_via_ctypes("/opt/axon/libaxon_pjrt.so")
    mod.get_axon_ntff_profile_hook = lambda: hook
    mod.set_axon_ntff_profile_hook = lambda h: None
    sys.modules["antenv.axon_hooks"] = mod
    import antenv

    antenv.axon_hooks = mod
    from concourse import bass_utils as _bu

    _bu.upload_artifacts = lambda tmpdir: tmpdir  # keep artifacts local


def kernel_profiled(**inputs):
    """Returns (output, exec_time_ns) using the NTFF profile of core 0."""
    _install_profile_hook()
    out, res = _run(inputs, trace=True)
    return out, res.exec_time_ns
